# revision 42
# baseline (speedup 1.0000x reference)
"""Trainium2 Bass kernel for a BasicTransformerBlock (self-attn + cross-attn + GEGLU FF).

Sharding: 8 cores = 2 batches x 4 sequence chunks of 1024 rows. Each core
redundantly computes LN1 + K/V projections over its batch's full 4096 rows
(position-independent, so all cores run an identical SPMD program) and
produces its own 1024-row slice of the output. No collectives.

Precision: fp32 residual stream and softmax statistics; bf16 weights and
activations for projections/FF; fp8e3m4 for the softmax probabilities and V
in the probability-x-V matmuls (errors there are diluted ~100x by the fp32
residual). Softmax runs without max-subtraction (scores are provably small
at this problem's scale: |s| < ~1.1) with 1/sqrt(dh) folded into the exp;
the denominator comes free from a ones-column in V.

All row-major -> feature-major transposes run on the DMA xbar engines
(single [128, 4x384] block transposes, issued from the ACT queue in the
LN1/KV phase and the SP queue elsewhere), keeping the PE array free for
matmuls; the softmax exp is split ~50/50 between ACT (hardware Exp) and the
vector engine (custom (1+z/32)^32 DVE op). The two q-halves are interleaved
per phase so attention-epilogue transpose latency hides under the other
half's matmul stream.
"""

import numpy as np
import ml_dtypes

DIM = 320
DIMP = 384  # DIM padded to a multiple of 128 for DMA-xbar transposes
HEADS = 8
DH = 40
CTX = 768
IFF = 1280  # GEGLU inner width; proj1 width = 2*IFF
EPS = 1e-5
SCALE = DH ** -0.5
NCORES = 8
MCTX = 77
VS = 336  # V row stride (8*41 = 328 padded to %16 for DoubleRow)

BF16 = ml_dtypes.bfloat16


def _chunks(total, step=128):
    out = []
    k = 0
    while k < total:
        out.append((k, min(step, total - k)))
        k += step
    return out


DIM_CHUNKS = _chunks(DIM)    # [(0,128),(128,128),(256,64)]
CTX_CHUNKS = _chunks(CTX)    # 6 x 128


def _register_exp_op():
    """Custom DVE op: out = (in0*s0 + s1)^32 — used as exp(z) ~ (1+z/32)^32
    to offload part of the softmax exp from ACT to the vector engine."""
    import concourse.dve_ops as dve_ops
    for o in dve_ops.OPS:
        if o.name == "EXP_POLY32_ANT":
            return o
    from concourse.dve_spec import Spec, Src0, C0, C1, sq
    spec = Spec(
        body=sq(sq(sq(sq(sq(Src0 * C0 + C1))))),
        reference=lambda in0, in1, s0, s1, imm2:
            ((in0.astype(np.float32) * s0 + s1) ** 32).astype(np.float32))
    op = dve_ops.DveOp("EXP_POLY32_ANT", spec, subdim=False,
                       uops_sha={"v3": "eafb894a1d5c531b"})
    dve_ops.OPS.append(op)
    dve_ops._SUB_OPCODE_FOR_NAME[op.name] = \
        dve_ops._CUSTOM_DVE_ROW_BASE + len(dve_ops.OPS) - 1
    dve_ops.CUSTOM_DVE_SPECS[op.name] = op.spec
    return op


def build_nc(S, R, flags=()):
    """Build + compile the per-core Bass program.

    flags: subset of {"ln1_w","ln1_b","ln2_w","ln2_b","ln3_w","ln3_b",
    "a1_bo","a2_bo","ff_b2"} that are non-trivial and must be applied.
    """
    import concourse.bass as bass
    import concourse.tile as tile
    from concourse import bacc, mybir
    from concourse.masks import make_identity

    f32 = mybir.dt.float32
    bf = mybir.dt.bfloat16
    f8 = mybir.dt.float8e3
    AF = mybir.ActivationFunctionType
    OP = mybir.AluOpType
    PM = mybir.MatmulPerfMode
    flags = set(flags)

    KB = S // 128     # key blocks (self-attn)
    QT = R // 128     # q row-tiles
    QHS = R // 512    # q 512-row groups

    nc = bacc.Bacc("TRN2", target_bir_lowering=False, debug=False)

    def din(name, shape, dt=bf):
        return nc.dram_tensor(name, shape, dt, kind="ExternalInput").ap()

    xfull_d = din("xfull", [S, DIMP])
    xq_d = din("xq", [R, DIMP], f32)
    ctxT_d = din("ctxT", [CTX, MCTX])
    w_d = {}
    for nm, shape in [
        ("a1_WqA", [DIM, 512]), ("a1_WqB", [DIM, 512]),
        ("a2_WqA", [DIM, 512]), ("a2_WqB", [DIM, 512]),
        ("a1_Wk", [DIM, 512]), ("a1_Wv", [DIM, DIM]),
        ("a1_Wo", [DIM, DIM]), ("a2_Wk", [CTX, 512]),
        ("a2_Wv", [CTX, DIM]), ("a2_Wo", [DIM, DIM]),
        ("ff_W1", [DIM, 2 * IFF]), ("ff_W2", [IFF, DIM]),
    ]:
        w_d[nm] = din(nm, shape)
    b1_d = din("ff_b1", [2 * IFF], f32)
    vec_d = {nm: din(nm, [DIM], f32) for nm in sorted(flags)}
    out_d = nc.dram_tensor("out", [R, DIM], f32, kind="ExternalOutput").ap()

    with tile.TileContext(nc) as tc:
        import contextlib
        with contextlib.ExitStack() as est:
            persist = est.enter_context(tc.tile_pool(name="persist", bufs=1))
            work = est.enter_context(tc.tile_pool(name="work", bufs=4))
            expp = est.enter_context(tc.tile_pool(name="expp", bufs=5))
            # One PSUM pool for the whole kernel: tag "sc" = 2 x [128,1024]f32
            # (4 banks), tag "acc" = 4 x [128,512]f32 (4 banks). All other
            # PSUM tiles allocate from these tags so phases can pipeline.
            psum = est.enter_context(tc.tile_pool(name="psum", bufs=2,
                                                  space="PSUM"))

            def ps_sc(shape, dt=f32, name="sc"):
                return psum.tile(shape, dt, tag="sc", bufs=2, name=name)

            def ps_acc(shape, dt=f32, name="accp"):
                return psum.tile(shape, dt, tag="acc", bufs=4, name=name)

            ident = persist.tile([128, 128], bf, name="ident")
            make_identity(nc, ident)
            eps_t = persist.tile([128, 1], f32, name="eps_t")
            nc.vector.memset(eps_t, EPS)

            # ---- persistent activations
            # h1T/actT are t-block-major [128, T, 3, 128]: each [p, t, :, :]
            # slice is per-partition contiguous (3*128), the layout the DMA
            # xbar transpose requires for its destination.
            h1T = persist.tile([128, S // 128, 3, 128], bf, name="h1T")
            Kf = persist.tile([128, 4, S], bf, name="Kf")        # 2-head blocks
            QfA = persist.tile([128, 4, R], bf, name="QfA")
            QfB = persist.tile([128, 4, R], bf, name="QfB")
            Vr = persist.tile([128, KB, VS], f8, name="Vr")
            K2f = persist.tile([128, 4, MCTX], bf, name="K2f")   # 2-head blocks
            # one q-half at a time (halves are processed sequentially)
            Q2fA = persist.tile([128, 4, 512], bf, name="Q2fA")
            Q2fB = persist.tile([128, 4, 512], bf, name="Q2fB")
            V2r = persist.tile([128, VS], f8, name="V2r")
            actT = persist.tile([128, QT, 3, 128], bf, name="actT")  # hqT/h2T/h3T
            resid = persist.tile([128, QT, DIMP], f32, name="resid")
            # Uff holds one q-half at a time (ff_out(qh) drains it before
            # ff_inner(qh+1) refills; PE program order enforces this anyway)
            Uff = persist.tile([128, IFF // 128, 512], bf, name="Uff")


            # ---- weights into SBUF, [in, out] layout chunked on partitions.
            # Loaded in stages so the critical path (xq/xfull -> LN1 -> Q/K/V)
            # is not queued behind 4.7MB of cross-attn/FF weights.
            wsb = {}

            def load_w(names):
                for nm in names:
                    chks = CTX_CHUNKS if nm in ("a2_Wk", "a2_Wv") else DIM_CHUNKS
                    width = w_d[nm].shape[1]
                    t = persist.tile([128, len(chks), width], bf, name=f"w_{nm}",
                                     uniquify=True)
                    for c, (k0, kw) in enumerate(chks):
                        nc.sync.dma_start(out=t[:kw, c, :],
                                          in_=w_d[nm][k0:k0 + kw, :])
                        if kw < 128:
                            # zero-pad so every chunk matmul runs with a full
                            # 128 contraction (uniform (128,128) tile mode —
                            # avoids PE mode-switch drains; extra rows hit
                            # zero weights and contraction depth is free)
                            nc.vector.memset(t[kw:128, c, :], 0.0)
                    wsb[nm] = t

            for t in range(QT):
                nc.sync.dma_start(out=resid[:, t, :],
                                  in_=xq_d[t * 128:(t + 1) * 128, :])
            load_w(["a1_WqA", "a1_WqB", "a1_Wk", "a1_Wv"])

            bcast = {}
            for nm in sorted(flags):
                t = persist.tile([128, DIM], f32, name=f"bc_{nm}")
                src = vec_d[nm]
                bc_ap = bass.AP(tensor=src.tensor, offset=src.offset,
                                ap=[[0, 128]] + [list(p) for p in src.ap])
                nc.gpsimd.dma_start(out=t, in_=bc_ap)
                bcast[nm] = t

            def ln_into(dst_bf, src_ap, wkey, bkey):
                stats = work.tile([128, 6], f32, tag="bnst", name="stats")
                nc.vector.bn_stats(stats, src_ap[:, 0:DIM])
                mv = work.tile([128, 2], f32, tag="bnagg", name="mv")
                nc.vector.bn_aggr(mv, stats)
                rstd = work.tile([128, 1], f32, tag="rstd", name="rstd")
                nc.scalar.activation(rstd, mv[:, 1:2], AF.Sqrt, bias=eps_t, scale=1.0)
                nc.vector.reciprocal(rstd, rstd)
                nc.vector.tensor_scalar(
                    out=dst_bf, in0=src_ap, scalar1=mv[:, 0:1], scalar2=rstd,
                    op0=OP.subtract, op1=OP.mult)
                if wkey in flags:
                    nc.vector.tensor_mul(out=dst_bf[:, 0:DIM],
                                         in0=dst_bf[:, 0:DIM], in1=bcast[wkey])
                if bkey in flags:
                    nc.vector.tensor_add(out=dst_bf[:, 0:DIM],
                                         in0=dst_bf[:, 0:DIM], in1=bcast[bkey])

            def transpose_blk(dstT4, src_blk, engine="sync"):
                """DMA-xbar transpose of a 4-tile [128, 4, DIMP] bf16 block
                into four per-partition-contiguous [128, 3, 128] feature-major
                blocks (dstT4 = [128, 4, 3, 128] slice). One instruction per
                512 rows; issued from SP or ACT to spread queue load."""
                eng = nc.sync if engine == "sync" else nc.scalar
                eng.dma_start_transpose(
                    out=dstT4.rearrange("p t d j -> p (t d) j"),
                    in_=src_blk.rearrange("p t f -> p (t f)"))

            def proj_fm(dst, wt, srcT, t_lo, t_hi, chks, copy_engine="dve",
                        tbase=0):
                """Feature-major projection via stationary (padded) weight cols.

                srcT is t-block-major [128, T, 3, 128]; processes rows
                t_lo*128 .. t_hi*128 in 512-wide groups (dst columns offset
                by tbase*128)."""
                for g in range(4):
                    for t0 in range(t_lo, t_hi, 4):
                        n0 = (t0 - tbase) * 128
                        ps = ps_acc([128, 512], name="proj_ps")
                        for c, (k0, kw) in enumerate(chks):
                            nc.tensor.matmul(
                                ps,
                                lhsT=wt[:128, c, 128 * g:128 * g + 128],
                                rhs=srcT[:128, t0:t0 + 4, c, :],
                                start=(c == 0), stop=(c == len(chks) - 1))
                        eng = copy_engine if copy_engine != "mix" else \
                            ("act" if (g + t0 // 4) % 2 == 0 else "dve")
                        if eng == "act":
                            nc.scalar.activation(dst[:, g, n0:n0 + 512], ps,
                                                 AF.Identity)
                        else:
                            nc.vector.tensor_copy(out=dst[:, g, n0:n0 + 512],
                                                  in_=ps)

            exp_op = _register_exp_op()

            def load_late_weights():
                load_w(["a1_Wo", "a2_WqA", "a2_WqB", "a2_Wk", "a2_Wv", "a2_Wo",
                        "ff_W1"])
                w2 = persist.tile([128, IFF // 128, DIM], bf, name="w_ff2")
                for c in range(IFF // 128):
                    nc.sync.dma_start(out=w2[:, c, :],
                                      in_=w_d["ff_W2"][c * 128:(c + 1) * 128, :])
                b1 = persist.tile([128, (2 * IFF) // 128], f32, name="b1t")
                nc.sync.dma_start(out=b1, in_=b1_d.rearrange("(c p) -> p c", p=128))
                ctxm = persist.tile([128, len(CTX_CHUNKS), MCTX], bf, name="ctxT_sb")
                for c, (k0, kw) in enumerate(CTX_CHUNKS):
                    nc.sync.dma_start(out=ctxm[:kw, c, :], in_=ctxT_d[k0:k0 + kw, :])
                return w2, b1, ctxm

            def cross_kv():
                for g in range(4):
                    ps = ps_sc([128, 128], name="k2_ps")
                    for c, (k0, kw) in enumerate(CTX_CHUNKS):
                        nc.tensor.matmul(
                            ps[:, :MCTX],
                            lhsT=wsb["a2_Wk"][:kw, c, 128 * g:128 * g + 128],
                            rhs=ctxT_sb[:kw, c, :],
                            start=(c == 0), stop=(c == len(CTX_CHUNKS) - 1))
                    nc.vector.tensor_copy(out=K2f[:, g, :], in_=ps[:, :MCTX])
                ps = ps_acc([128, 512], name="v2_ps")
                for c, (k0, kw) in enumerate(CTX_CHUNKS):
                    nc.tensor.matmul(
                        ps[:MCTX, :DIM], lhsT=ctxT_sb[:kw, c, :],
                        rhs=wsb["a2_Wv"][:kw, c, :],
                        start=(c == 0), stop=(c == len(CTX_CHUNKS) - 1))
                nc.vector.tensor_copy(
                    out=V2r[:MCTX, 0:328].rearrange("p (h c) -> p h c", c=41)[:, :, 0:40],
                    in_=ps[:MCTX, :DIM].rearrange("p (h c) -> p h c", c=40))
                nc.vector.memset(
                    V2r[:MCTX, 0:328].rearrange("p (h c) -> p h c",
                                                c=41)[:, :, 40:41], 1.0)

            # ---- own rows first: LN1 -> hqT, Qf (so attention can start as
            # soon as the leading K/V blocks exist; xq was DMA'd first above)
            for qb in range(QT // 4):
                hb = work.tile([128, 4, DIMP], bf, tag="hblk", bufs=2, name="hq")
                for tt in range(4):
                    t = qb * 4 + tt
                    ln_into(hb[:, tt, :], resid[:, t, :], "ln1_w", "ln1_b")
                transpose_blk(actT[:, qb * 4:qb * 4 + 4, :, :], hb,
                              "scalar" if qb % 2 == 0 else "sync")
            proj_fm(QfA, wsb["a1_WqA"], actT, 0, QT, DIM_CHUNKS)
            proj_fm(QfB, wsb["a1_WqB"], actT, 0, QT, DIM_CHUNKS)

            # ---- attn1 building blocks
            def attn1_scores_exp(q0, hp, kb):
                sc = ps_sc([128, 1024], name="sc")
                # full-128 contraction with ONE shared stationary (K group
                # hp): Qf{A,B} hold only the even/odd head at its partition
                # offset with exact zeros elsewhere, so the other head's K
                # rows contribute nothing. Keeps all of attn1 in (128,128)
                # tile mode (no PE mode-switch drains) and halves score LDWs.
                kblk = Kf[:, hp, kb * 128:(kb + 1) * 128]
                nc.tensor.matmul(sc[:, 0:512], lhsT=kblk,
                                 rhs=QfA[:, hp, q0:q0 + 512],
                                 start=True, stop=True)
                nc.tensor.matmul(sc[:, 512:1024], lhsT=kblk,
                                 rhs=QfB[:, hp, q0:q0 + 512],
                                 start=True, stop=True)
                ep = expp.tile([128, 1024], f8, tag="ep", name="ep")
                # each tile's exp split across ACT and DVE: halves the exp
                # latency in the scores->exp->sc-slot chain that paces attn1
                # (PE has idle headroom here since the (128,128) rework)
                nc.scalar.activation(ep[:, 0:512], sc[:, 0:512], AF.Exp,
                                     scale=SCALE)
                nc.vector._custom_dve(exp_op, out=ep[:, 512:1024],
                                      in0=sc[:, 512:1024],
                                      s0=SCALE / 32.0, s1=1.0)
                return ep

            def attn1_pv(acc, hp, kb, ep):
                for j in range(2):
                    hh = 2 * hp + j
                    for qs in range(4):
                        nc.tensor.matmul(
                            acc[qs][:, 41 * hh:41 * hh + 41],
                            lhsT=ep[:, j * 512 + qs * 128:j * 512 + (qs + 1) * 128],
                            rhs=Vr[:, kb, 41 * hh:41 * hh + 41],
                            start=(kb == 0), stop=(kb == KB - 1),
                            skip_group_check=True)

            # ---- LN1 + K/V production, merged with attn1 half-0.
            # Block nb's LN/transpose/K/V-proj items are interleaved ~1:1.2
            # with block (nb-1)'s score items, so the exp latency of half-0
            # hides under proj PE work (and vice versa). Scores and K/V proj
            # share the "sc" PSUM tag; the half-0 PV accumulators hold the
            # "acc" tag throughout.
            xts = {}

            def load_xt(t):
                xt = work.tile([128, DIMP], bf, tag="xt", bufs=5, name="xt")
                nc.sync.dma_start(out=xt, in_=xfull_d[t * 128:(t + 1) * 128, :])
                xts[t] = xt

            def kproj_item(nb, g):
                ps = ps_acc([128, 512], name="kf_ps")
                for c, (k0, kw) in enumerate(DIM_CHUNKS):
                    nc.tensor.matmul(
                        ps,
                        lhsT=wsb["a1_Wk"][:128, c, 128 * g:128 * g + 128],
                        rhs=h1T[:128, nb * 4:(nb + 1) * 4, c, :],
                        start=(c == 0), stop=(c == len(DIM_CHUNKS) - 1))
                nc.scalar.activation(Kf[:, g, nb * 512:(nb + 1) * 512], ps,
                                     AF.Identity)

            def vproj_item(t):
                ps = ps_acc([128, 512], name="v_ps")
                for c, (k0, kw) in enumerate(DIM_CHUNKS):
                    nc.tensor.matmul(
                        ps[:, :DIM],
                        lhsT=h1T[:128, t, c, :],
                        rhs=wsb["a1_Wv"][:128, c, :],
                        start=(c == 0), stop=(c == len(DIM_CHUNKS) - 1))
                nc.vector.tensor_copy(
                    out=Vr[:, t, 0:328].rearrange("p (h c) -> p h c",
                                                  c=41)[:, :, 0:40],
                    in_=ps[:, :DIM].rearrange("p (h c) -> p h c", c=40))

            # ones-denominator columns for ALL kb blocks, written once before
            # any V data or PV read (vproj writes never touch column 40)
            nc.vector.memset(
                Vr[:, 0:KB, 0:328].rearrange(
                    "p b (h c) -> p b h c", c=41)[:, :, :, 40], 1.0)
            for t in range(4):
                load_xt(t)

            # ---- shared attention epilogue: normalize, transpose, proj, add
            def finish_attn_norm(acc):
                """Normalize the PV accumulators into an SBUF block and issue
                its transpose; reads (and thus frees) the acc PSUM banks."""
                ab = work.tile([128, 4, DIMP], bf, tag="armblk", bufs=1,
                               name="armblk")
                nc.vector.memset(ab[:, :, DIM:DIMP], 0.0)
                for qs in range(4):
                    rec = work.tile([128, HEADS], f32, tag="rec", name="rec")
                    nc.vector.reciprocal(
                        rec, acc[qs].rearrange("p (h c) -> p h c", c=41)[:, :, 40])
                    rb = bass.AP(tensor=rec.tensor, offset=rec.offset,
                                 ap=[list(rec.ap[0]), [rec.ap[1][0], HEADS],
                                     [0, 40]])
                    nc.vector.tensor_mul(
                        out=ab[:, qs, 0:DIM].rearrange("p (h c) -> p h c", c=40),
                        in0=acc[qs].rearrange("p (h c) -> p h c", c=41)[:, :, 0:40],
                        in1=rb)
                afm = work.tile([128, 4, 3, 128], bf, tag="afm", bufs=2,
                                name="afm")
                transpose_blk(afm, ab)
                return afm

            def finish_attn_proj(qh, afm, wo, bo_key):
                for qs in range(4):
                    po = ps_acc([128, DIM], name="po")
                    for c, (k0, kw) in enumerate(DIM_CHUNKS):
                        nc.tensor.matmul(po, lhsT=afm[:128, qs, c, :],
                                         rhs=wo[:128, c, :],
                                         start=(c == 0),
                                         stop=(c == len(DIM_CHUNKS) - 1))
                    t = qh * 4 + qs
                    nc.vector.tensor_add(out=resid[:, t, 0:DIM],
                                         in0=resid[:, t, 0:DIM], in1=po)
                    if bo_key in flags:
                        nc.vector.tensor_add(out=resid[:, t, 0:DIM],
                                             in0=resid[:, t, 0:DIM],
                                             in1=bcast[bo_key])

            NMT = (2 * IFF) // 128  # 20

            def attn1_half(qh):
                """Self-attention scores+PV for one q-half (PV pipelined two
                tiles back). Returns the acc PSUM tiles."""
                q0 = qh * 512
                acc = [ps_acc([128, HEADS * 41], name=f"acc{qs}")
                       for qs in range(4)]
                pending = []
                for hp in range(HEADS // 2):
                    for kb in range(KB):
                        ep = attn1_scores_exp(q0, hp, kb)
                        pending.append((hp, kb, ep))
                        if kb % 4 == 3:
                            while len(pending) > 2:
                                attn1_pv(acc, *pending.pop(0))
                for phk in pending:
                    attn1_pv(acc, *phk)
                return acc

            def attn1_half0_merged():
                """KV production for block nb interleaved with block (nb-1)'s
                half-0 score/exp/PV items."""
                acc = [ps_acc([128, HEADS * 41], name=f"acc{qs}")
                       for qs in range(4)]
                pending = []

                def score_item(hp, kb):
                    ep = attn1_scores_exp(0, hp, kb)
                    pending.append((hp, kb, ep))
                    if len(pending) > 2:
                        attn1_pv(acc, *pending.pop(0))

                NB = S // 512
                for nb in range(NB + 1):
                    thunks = []
                    if nb < NB:
                        hb = work.tile([128, 4, DIMP], bf, tag="hblk", bufs=2,
                                       name="h1")
                        for tt in range(4):
                            t = nb * 4 + tt
                            if t + 4 < S // 128:
                                load_xt(t + 4)
                            thunks.append((ln_into, hb[:, tt, :], xts.pop(t),
                                           "ln1_w", "ln1_b"))
                        thunks.append((transpose_blk,
                                       h1T[:, nb * 4:nb * 4 + 4, :, :], hb,
                                       "scalar" if nb % 2 == 0 else "sync"))
                        for g in range(4):
                            thunks.append((kproj_item, nb, g))
                        for tt in range(4):
                            thunks.append((vproj_item, nb * 4 + tt))
                    sitems = []
                    if nb > 0:
                        for kb in range((nb - 1) * 4, nb * 4):
                            for hp in range(4):
                                sitems.append((hp, kb))
                    si = 0
                    for i, th in enumerate(thunks):
                        th[0](*th[1:])
                        tgt = (len(sitems) * (i + 1)) // max(len(thunks), 1)
                        while si < min(tgt, len(sitems)):
                            score_item(*sitems[si])
                            si += 1
                    while si < len(sitems):
                        score_item(*sitems[si])
                        si += 1
                while pending:
                    attn1_pv(acc, *pending.pop(0))
                return acc

            def ln_block(qh, wkey, bkey):
                hb = work.tile([128, 4, DIMP], bf, tag="hblk", bufs=2, name="hb")
                for tt in range(4):
                    t = qh * 4 + tt
                    ln_into(hb[:, tt, :], resid[:, t, :], wkey, bkey)
                transpose_blk(actT[:, qh * 4:qh * 4 + 4, :, :], hb)

            def attn2_half(qh):
                q0 = qh * 512
                proj_fm(Q2fA, wsb["a2_WqA"], actT, qh * 4, qh * 4 + 4,
                        DIM_CHUNKS, tbase=qh * 4)
                proj_fm(Q2fB, wsb["a2_WqB"], actT, qh * 4, qh * 4 + 4,
                        DIM_CHUNKS, tbase=qh * 4)
                acc = [ps_acc([128, HEADS * 41], name=f"acc2_{qs}")
                       for qs in range(4)]
                p2 = []
                for hp in range(HEADS // 2):
                    sc = ps_sc([128, 1024], name="sc2")
                    k2blk = K2f[:, hp, :]
                    nc.tensor.matmul(sc[:MCTX, 0:512], lhsT=k2blk,
                                     rhs=Q2fA[:, hp, 0:512],
                                     start=True, stop=True)
                    nc.tensor.matmul(sc[:MCTX, 512:1024], lhsT=k2blk,
                                     rhs=Q2fB[:, hp, 0:512],
                                     start=True, stop=True)
                    ep = expp.tile([128, 1024], f8, tag="ep2", bufs=2, name="ep2")
                    if hp % 2 == 1:
                        nc.vector._custom_dve(exp_op, out=ep[:MCTX, :],
                                              in0=sc[:MCTX, :],
                                              s0=SCALE / 32.0, s1=1.0)
                    else:
                        nc.scalar.activation(ep[:MCTX, :], sc[:MCTX, :], AF.Exp,
                                             scale=SCALE)
                    p2.append((hp, ep))
                for hp, ep in p2:
                    for j in range(2):
                        hh = 2 * hp + j
                        for qs in range(4):
                            nc.tensor.matmul(
                                acc[qs][:, 41 * hh:41 * hh + 41],
                                lhsT=ep[:MCTX, j * 512 + qs * 128:
                                        j * 512 + (qs + 1) * 128],
                                rhs=V2r[:MCTX, 41 * hh:41 * hh + 41],
                                start=True, stop=True, skip_group_check=True)
                return acc

            def ff_inner(qh):
                _order = [m for pair in zip(range(NMT // 2), range(NMT // 2, NMT))
                          for m in pair]
                for mt in _order:
                    ps = ps_acc([128, 512], name="ff1_ps")
                    for c, (k0, kw) in enumerate(DIM_CHUNKS):
                        nc.tensor.matmul(
                            ps, lhsT=wsb["ff_W1"][:128, c, mt * 128:(mt + 1) * 128],
                            rhs=actT[:128, qh * 4:qh * 4 + 4, c, :],
                            start=(c == 0), stop=(c == len(DIM_CHUNKS) - 1))
                    if mt < NMT // 2:
                        nc.scalar.activation(Uff[:, mt, :], ps,
                                             AF.Identity,
                                             bias=b1t[:, mt:mt + 1], scale=1.0)
                    else:
                        gl = work.tile([128, 512], bf, tag="gel", name="gel")
                        nc.scalar.activation(gl, ps, AF.Gelu,
                                             bias=b1t[:, mt:mt + 1], scale=1.0)
                        mu = mt - NMT // 2
                        nc.vector.tensor_mul(out=Uff[:, mu, :],
                                             in0=Uff[:, mu, :], in1=gl)

            def ff_out(qh):
                for tt in range(4):
                    qs = qh * 4 + tt
                    po = ps_acc([128, DIM], name="ff2_ps")
                    for c in range(IFF // 128):
                        nc.tensor.matmul(po,
                                         lhsT=Uff[:, c, tt * 128:(tt + 1) * 128],
                                         rhs=w2_sb[:, c, :],
                                         start=(c == 0), stop=(c == IFF // 128 - 1))
                    ot = work.tile([128, DIM], f32, tag="ot", name="ot")
                    nc.vector.tensor_add(out=ot, in0=resid[:, qs, 0:DIM], in1=po)
                    if "ff_b2" in flags:
                        nc.vector.tensor_add(out=ot, in0=ot, in1=bcast["ff_b2"])
                    nc.sync.dma_start(out=out_d[qs * 128:(qs + 1) * 128, :], in_=ot)

            # ============ emission schedule: the two q-halves are interleaved
            # within each phase so every finish/LN transpose's DMA latency is
            # covered by the other half's matmul stream.
            NB = S // 512
            for nb in range(NB):
                hb = work.tile([128, 4, DIMP], bf, tag="hblk", bufs=2, name="h1")
                for tt in range(4):
                    t = nb * 4 + tt
                    if t + 4 < S // 128:
                        load_xt(t + 4)
                    ln_into(hb[:, tt, :], xts.pop(t), "ln1_w", "ln1_b")
                transpose_blk(h1T[:, nb * 4:nb * 4 + 4, :, :], hb,
                              "scalar" if nb % 2 == 0 else "sync")
                for g in range(4):
                    kproj_item(nb, g)
                for tt in range(4):
                    vproj_item(nb * 4 + tt)
            acc = attn1_half(0)
            afm0 = finish_attn_norm(acc)          # frees acc banks for qh1
            # cross-attn/FF weights + context K,V hide under attn1(qh0)/(qh1)
            w2_sb, b1t, ctxT_sb = load_late_weights()
            cross_kv()
            acc = attn1_half(1)
            afm1 = finish_attn_norm(acc)
            finish_attn_proj(0, afm0, wsb["a1_Wo"], "a1_bo")
            ln_block(0, "ln2_w", "ln2_b")
            finish_attn_proj(1, afm1, wsb["a1_Wo"], "a1_bo")
            acc = attn2_half(0)
            afm0 = finish_attn_norm(acc)
            ln_block(1, "ln2_w", "ln2_b")
            finish_attn_proj(0, afm0, wsb["a2_Wo"], "a2_bo")
            acc = attn2_half(1)
            afm1 = finish_attn_norm(acc)
            ln_block(0, "ln3_w", "ln3_b")
            finish_attn_proj(1, afm1, wsb["a2_Wo"], "a2_bo")
            ff_inner(0)
            ln_block(1, "ln3_w", "ln3_b")
            ff_out(0)
            ff_inner(1)
            ff_out(1)

    nc.compile()
    return nc


_CACHE = {}


def _get_nc(S, R, flags):
    key = (S, R, tuple(sorted(flags)))
    if key not in _CACHE:
        _CACHE[key] = build_nc(S, R, flags)
    return _CACHE[key]


def _pad_qk8(w):
    """Self-attn Q/K weight layout for fp8 DoubleRow: per head h (g=h//4,
    m=h%4), sub i: block col 128*(2g+i) + 32*m + dk <- w col 40h + 20i + dk."""
    w = np.asarray(w)
    out = np.zeros((w.shape[0], 512), w.dtype)
    for h in range(HEADS):
        g, m = divmod(h, 4)
        for i in range(2):
            c0 = 128 * (2 * g + i) + 32 * m
            out[:, c0:c0 + 20] = w[:, DH * h + 20 * i:DH * h + 20 * i + 20]
    return out


def _pad_qk2(w, par=None):
    """Q/K layout: 2-head groups at partition offsets {0,64}. With par set,
    only even (par=0) or odd (par=1) heads are kept (others zero) so the
    score matmul can contract over all 128 partitions with one shared K."""
    w = np.asarray(w)
    out = np.zeros((w.shape[0], 512), w.dtype)
    for h in range(HEADS):
        g, j = divmod(h, 2)
        if par is not None and j != par:
            continue
        out[:, 128 * g + 64 * j:128 * g + 64 * j + DH] = w[:, DH * h:DH * h + DH]
    return out


def make_in_maps(x, context, ln_params, weights):
    """Host-side prep: returns (flags, in_maps, R, S, Bn)."""
    x = np.asarray(x)
    context = np.asarray(context)
    Bn = x.shape[0]
    S = x.shape[1]
    R = S * Bn // NCORES
    flags = set()
    for nm in ("ln1_w", "ln2_w", "ln3_w"):
        if not np.allclose(np.asarray(ln_params[nm]), 1.0):
            flags.add(nm)
    for nm in ("ln1_b", "ln2_b", "ln3_b", "a1_bo", "a2_bo", "ff_b2"):
        if not np.allclose(np.asarray(ln_params[nm]), 0.0):
            flags.add(nm)
    weights = dict(weights)
    w1q = weights.pop("a1_Wq")
    weights["a1_WqA"] = _pad_qk2(w1q, 0)
    weights["a1_WqB"] = _pad_qk2(w1q, 1)
    weights["a1_Wk"] = _pad_qk2(weights["a1_Wk"])
    w2q = weights.pop("a2_Wq")
    weights["a2_WqA"] = _pad_qk2(w2q, 0)
    weights["a2_WqB"] = _pad_qk2(w2q, 1)
    weights["a2_Wk"] = _pad_qk2(weights["a2_Wk"])
    shared = {nm: np.ascontiguousarray(np.asarray(w).astype(BF16))
              for nm, w in weights.items()}
    shared["ff_b1"] = np.ascontiguousarray(
        np.asarray(ln_params["ff_b1"]).astype(np.float32))
    for nm in flags:
        shared[nm] = np.ascontiguousarray(
            np.asarray(ln_params[nm]).astype(np.float32))
    pad = ((0, 0), (0, 0), (0, DIMP - DIM))
    xbf = np.ascontiguousarray(np.pad(x, pad).astype(BF16))
    ctxT = np.ascontiguousarray(np.asarray(context).astype(BF16).transpose(0, 2, 1))
    xf32 = np.ascontiguousarray(np.pad(x, pad).astype(np.float32))
    in_maps = []
    cpb = NCORES // Bn
    for core in range(NCORES):
        b, c = divmod(core, cpb)
        m = dict(shared)
        m["xfull"] = xbf[b]
        m["xq"] = np.ascontiguousarray(xf32[b, c * R:(c + 1) * R])
        m["ctxT"] = ctxT[b]
        in_maps.append(m)
    return flags, in_maps, R, S, Bn


def kernel(x, context, ln1_w, ln1_b, ln2_w, ln2_b, ln3_w, ln3_b,
           a1_Wq, a1_Wk, a1_Wv, a1_Wo, a1_bo,
           a2_Wq, a2_Wk, a2_Wv, a2_Wo, a2_bo,
           ff_W1, ff_b1, ff_W2, ff_b2, _trace=False):
    from concourse.bass_utils import run_bass_kernel_spmd

    weights = dict(a1_Wq=a1_Wq, a1_Wk=a1_Wk, a1_Wv=a1_Wv, a1_Wo=a1_Wo,
                   a2_Wq=a2_Wq, a2_Wk=a2_Wk, a2_Wv=a2_Wv, a2_Wo=a2_Wo,
                   ff_W1=ff_W1, ff_W2=ff_W2)
    ln_params = dict(ln1_w=ln1_w, ln1_b=ln1_b, ln2_w=ln2_w, ln2_b=ln2_b,
                     ln3_w=ln3_w, ln3_b=ln3_b, a1_bo=a1_bo, a2_bo=a2_bo,
                     ff_b1=ff_b1, ff_b2=ff_b2)
    flags, in_maps, R, S, Bn = make_in_maps(x, context, ln_params, weights)
    nc = _get_nc(S, R, flags)
    res = run_bass_kernel_spmd(nc, in_maps, core_ids=list(range(NCORES)),
                               trace=_trace)
    out = np.empty((Bn, S, DIM), np.float32)
    cpb = NCORES // Bn
    for core in range(NCORES):
        b, c = divmod(core, cpb)
        out[b, c * R:(c + 1) * R] = res.results[core]["out"]
    kernel._last_result = res
    return out



# revision 43
# speedup vs baseline: 1.0476x; 1.0476x over previous
"""Trainium2 Bass kernel for a BasicTransformerBlock (self-attn + cross-attn + GEGLU FF).

Sharding: 8 cores = 2 batches x 4 sequence chunks of 1024 rows. Each core
redundantly computes LN1 + K/V projections over its batch's full 4096 rows
(position-independent, so all cores run an identical SPMD program) and
produces its own 1024-row slice of the output. No collectives.

Precision: fp32 residual stream and softmax statistics; bf16 weights and
activations for projections/FF; fp8e3m4 for the softmax probabilities and V
in the probability-x-V matmuls (errors there are diluted ~100x by the fp32
residual). Softmax runs without max-subtraction (scores are provably small
at this problem's scale: |s| < ~1.1) with 1/sqrt(dh) folded into the exp;
the denominator comes free from a ones-column in V.

All row-major -> feature-major transposes run on the DMA xbar engines
(single [128, 4x384] block transposes, issued from the ACT queue in the
LN1/KV phase and the SP queue elsewhere), keeping the PE array free for
matmuls; the softmax exp is split ~50/50 between ACT (hardware Exp) and the
vector engine (custom (1+z/32)^32 DVE op). The two q-halves are interleaved
per phase so attention-epilogue transpose latency hides under the other
half's matmul stream.
"""

import numpy as np
import ml_dtypes

DIM = 320
DIMP = 384  # DIM padded to a multiple of 128 for DMA-xbar transposes
HEADS = 8
DH = 40
CTX = 768
IFF = 1280  # GEGLU inner width; proj1 width = 2*IFF
EPS = 1e-5
SCALE = DH ** -0.5
NCORES = 8
MCTX = 77
VS = 336  # V row stride (8*41 = 328 padded to %16 for DoubleRow)

BF16 = ml_dtypes.bfloat16


def _chunks(total, step=128):
    out = []
    k = 0
    while k < total:
        out.append((k, min(step, total - k)))
        k += step
    return out


DIM_CHUNKS = _chunks(DIM)    # [(0,128),(128,128),(256,64)]
CTX_CHUNKS = _chunks(CTX)    # 6 x 128


def _register_exp_op():
    """Custom DVE op: out = (in0*s0 + s1)^32 — used as exp(z) ~ (1+z/32)^32
    to offload part of the softmax exp from ACT to the vector engine."""
    import concourse.dve_ops as dve_ops
    for o in dve_ops.OPS:
        if o.name == "EXP_POLY32_ANT":
            return o
    from concourse.dve_spec import Spec, Src0, C0, C1, sq
    spec = Spec(
        body=sq(sq(sq(sq(sq(Src0 * C0 + C1))))),
        reference=lambda in0, in1, s0, s1, imm2:
            ((in0.astype(np.float32) * s0 + s1) ** 32).astype(np.float32))
    op = dve_ops.DveOp("EXP_POLY32_ANT", spec, subdim=False,
                       uops_sha={"v3": "eafb894a1d5c531b"})
    dve_ops.OPS.append(op)
    dve_ops._SUB_OPCODE_FOR_NAME[op.name] = \
        dve_ops._CUSTOM_DVE_ROW_BASE + len(dve_ops.OPS) - 1
    dve_ops.CUSTOM_DVE_SPECS[op.name] = op.spec
    return op


def build_nc(S, R, flags=()):
    """Build + compile the per-core Bass program.

    flags: subset of {"ln1_w","ln1_b","ln2_w","ln2_b","ln3_w","ln3_b",
    "a1_bo","a2_bo","ff_b2"} that are non-trivial and must be applied.
    """
    import concourse.bass as bass
    import concourse.tile as tile
    from concourse import bacc, mybir
    from concourse.masks import make_identity

    f32 = mybir.dt.float32
    bf = mybir.dt.bfloat16
    f8 = mybir.dt.float8e3
    AF = mybir.ActivationFunctionType
    OP = mybir.AluOpType
    PM = mybir.MatmulPerfMode
    flags = set(flags)

    KB = S // 128     # key blocks (self-attn)
    QT = R // 128     # q row-tiles
    QHS = R // 512    # q 512-row groups

    nc = bacc.Bacc("TRN2", target_bir_lowering=False, debug=False)

    def din(name, shape, dt=bf):
        return nc.dram_tensor(name, shape, dt, kind="ExternalInput").ap()

    xfull_d = din("xfull", [S, DIMP])
    xq_d = din("xq", [R, DIMP], f32)
    ctxT_d = din("ctxT", [CTX, MCTX])
    w_d = {}
    for nm, shape in [
        ("a1_WqA", [DIM, 512]), ("a1_WqB", [DIM, 512]),
        ("a2_WqA", [DIM, 512]), ("a2_WqB", [DIM, 512]),
        ("a1_Wk", [DIM, 512]), ("a1_Wv", [DIM, DIM]),
        ("a1_Wo", [DIM, DIM]), ("a2_Wk", [CTX, 512]),
        ("a2_Wv", [CTX, DIM]), ("a2_Wo", [DIM, DIM]),
        ("ff_W1", [DIM, 2 * IFF]), ("ff_W2", [IFF, DIM]),
    ]:
        w_d[nm] = din(nm, shape)
    b1_d = din("ff_b1", [2 * IFF], f32)
    vec_d = {nm: din(nm, [DIM], f32) for nm in sorted(flags)}
    out_d = nc.dram_tensor("out", [R, DIM], f32, kind="ExternalOutput").ap()

    with tile.TileContext(nc) as tc:
        import contextlib
        with contextlib.ExitStack() as est:
            persist = est.enter_context(tc.tile_pool(name="persist", bufs=1))
            work = est.enter_context(tc.tile_pool(name="work", bufs=4))
            expp = est.enter_context(tc.tile_pool(name="expp", bufs=5))
            # One PSUM pool for the whole kernel: tag "sc" = 2 x [128,1024]f32
            # (4 banks), tag "acc" = 4 x [128,512]f32 (4 banks). All other
            # PSUM tiles allocate from these tags so phases can pipeline.
            psum = est.enter_context(tc.tile_pool(name="psum", bufs=2,
                                                  space="PSUM"))

            def ps_sc(shape, dt=f32, name="sc"):
                return psum.tile(shape, dt, tag="sc", bufs=2, name=name)

            def ps_acc(shape, dt=f32, name="accp"):
                return psum.tile(shape, dt, tag="acc", bufs=4, name=name)

            ident = persist.tile([128, 128], bf, name="ident")
            make_identity(nc, ident)
            eps_t = persist.tile([128, 1], f32, name="eps_t")
            nc.vector.memset(eps_t, EPS)

            # ---- persistent activations
            # h1T/actT are t-block-major [128, T, 3, 128]: each [p, t, :, :]
            # slice is per-partition contiguous (3*128), the layout the DMA
            # xbar transpose requires for its destination.
            h1T = persist.tile([128, S // 128, 3, 128], bf, name="h1T")
            Kf = persist.tile([128, 4, S], bf, name="Kf")        # 2-head blocks
            QfA = persist.tile([128, 4, R], bf, name="QfA")
            QfB = persist.tile([128, 4, R], bf, name="QfB")
            Vr = persist.tile([128, KB, VS], f8, name="Vr")
            K2f = persist.tile([128, 4, MCTX], bf, name="K2f")   # 2-head blocks
            # one q-half at a time (halves are processed sequentially)
            Q2fA = persist.tile([128, 4, 512], bf, name="Q2fA")
            Q2fB = persist.tile([128, 4, 512], bf, name="Q2fB")
            V2r = persist.tile([128, VS], f8, name="V2r")
            actT = persist.tile([128, QT, 3, 128], bf, name="actT")  # hqT/h2T/h3T
            resid = persist.tile([128, QT, DIMP], f32, name="resid")
            # Uff holds one q-half at a time (ff_out(qh) drains it before
            # ff_inner(qh+1) refills; PE program order enforces this anyway)
            Uff = persist.tile([128, IFF // 128, 512], bf, name="Uff")


            # ---- weights into SBUF, [in, out] layout chunked on partitions.
            # Loaded in stages so the critical path (xq/xfull -> LN1 -> Q/K/V)
            # is not queued behind 4.7MB of cross-attn/FF weights.
            wsb = {}

            def load_w(names):
                for nm in names:
                    chks = CTX_CHUNKS if nm in ("a2_Wk", "a2_Wv") else DIM_CHUNKS
                    width = w_d[nm].shape[1]
                    t = persist.tile([128, len(chks), width], bf, name=f"w_{nm}",
                                     uniquify=True)
                    for c, (k0, kw) in enumerate(chks):
                        nc.sync.dma_start(out=t[:kw, c, :],
                                          in_=w_d[nm][k0:k0 + kw, :])
                        if kw < 128:
                            # zero-pad so every chunk matmul runs with a full
                            # 128 contraction (uniform (128,128) tile mode —
                            # avoids PE mode-switch drains; extra rows hit
                            # zero weights and contraction depth is free)
                            nc.vector.memset(t[kw:128, c, :], 0.0)
                    wsb[nm] = t

            for t in range(QT):
                nc.sync.dma_start(out=resid[:, t, :],
                                  in_=xq_d[t * 128:(t + 1) * 128, :])
            load_w(["a1_WqA", "a1_WqB", "a1_Wk", "a1_Wv"])

            bcast = {}
            for nm in sorted(flags):
                t = persist.tile([128, DIM], f32, name=f"bc_{nm}")
                src = vec_d[nm]
                bc_ap = bass.AP(tensor=src.tensor, offset=src.offset,
                                ap=[[0, 128]] + [list(p) for p in src.ap])
                nc.gpsimd.dma_start(out=t, in_=bc_ap)
                bcast[nm] = t

            def ln_into(dst_bf, src_ap, wkey, bkey):
                stats = work.tile([128, 6], f32, tag="bnst", name="stats")
                nc.vector.bn_stats(stats, src_ap[:, 0:DIM])
                mv = work.tile([128, 2], f32, tag="bnagg", name="mv")
                nc.vector.bn_aggr(mv, stats)
                rstd = work.tile([128, 1], f32, tag="rstd", name="rstd")
                nc.scalar.activation(rstd, mv[:, 1:2], AF.Sqrt, bias=eps_t, scale=1.0)
                nc.vector.reciprocal(rstd, rstd)
                nc.vector.tensor_scalar(
                    out=dst_bf, in0=src_ap, scalar1=mv[:, 0:1], scalar2=rstd,
                    op0=OP.subtract, op1=OP.mult)
                if wkey in flags:
                    nc.vector.tensor_mul(out=dst_bf[:, 0:DIM],
                                         in0=dst_bf[:, 0:DIM], in1=bcast[wkey])
                if bkey in flags:
                    nc.vector.tensor_add(out=dst_bf[:, 0:DIM],
                                         in0=dst_bf[:, 0:DIM], in1=bcast[bkey])

            def transpose_blk(dstT4, src_blk, engine="sync"):
                """DMA-xbar transpose of a 4-tile [128, 4, DIMP] bf16 block
                into four per-partition-contiguous [128, 3, 128] feature-major
                blocks (dstT4 = [128, 4, 3, 128] slice). One instruction per
                512 rows; issued from SP or ACT to spread queue load."""
                eng = nc.sync if engine == "sync" else nc.scalar
                eng.dma_start_transpose(
                    out=dstT4.rearrange("p t d j -> p (t d) j"),
                    in_=src_blk.rearrange("p t f -> p (t f)"))

            def proj_fm(dst, wt, srcT, t_lo, t_hi, chks, copy_engine="dve",
                        tbase=0):
                """Feature-major projection via stationary (padded) weight cols.

                srcT is t-block-major [128, T, 3, 128]; processes rows
                t_lo*128 .. t_hi*128 in 512-wide groups (dst columns offset
                by tbase*128)."""
                for g in range(4):
                    for t0 in range(t_lo, t_hi, 4):
                        n0 = (t0 - tbase) * 128
                        ps = ps_acc([128, 512], name="proj_ps")
                        for c, (k0, kw) in enumerate(chks):
                            nc.tensor.matmul(
                                ps,
                                lhsT=wt[:128, c, 128 * g:128 * g + 128],
                                rhs=srcT[:128, t0:t0 + 4, c, :],
                                start=(c == 0), stop=(c == len(chks) - 1))
                        eng = copy_engine if copy_engine != "mix" else \
                            ("act" if (g + t0 // 4) % 2 == 0 else "dve")
                        if eng == "act":
                            nc.scalar.activation(dst[:, g, n0:n0 + 512], ps,
                                                 AF.Identity)
                        else:
                            nc.vector.tensor_copy(out=dst[:, g, n0:n0 + 512],
                                                  in_=ps)

            exp_op = _register_exp_op()

            def load_late_weights():
                load_w(["a1_Wo", "a2_WqA", "a2_WqB", "a2_Wk", "a2_Wv", "a2_Wo",
                        "ff_W1"])
                w2 = persist.tile([128, IFF // 128, DIM], bf, name="w_ff2")
                for c in range(IFF // 128):
                    nc.sync.dma_start(out=w2[:, c, :],
                                      in_=w_d["ff_W2"][c * 128:(c + 1) * 128, :])
                b1 = persist.tile([128, (2 * IFF) // 128], f32, name="b1t")
                nc.sync.dma_start(out=b1, in_=b1_d.rearrange("(c p) -> p c", p=128))
                ctxm = persist.tile([128, len(CTX_CHUNKS), MCTX], bf, name="ctxT_sb")
                for c, (k0, kw) in enumerate(CTX_CHUNKS):
                    nc.sync.dma_start(out=ctxm[:kw, c, :], in_=ctxT_d[k0:k0 + kw, :])
                return w2, b1, ctxm

            def cross_kv():
                for g in range(4):
                    ps = ps_sc([128, 128], name="k2_ps")
                    for c, (k0, kw) in enumerate(CTX_CHUNKS):
                        nc.tensor.matmul(
                            ps[:, :MCTX],
                            lhsT=wsb["a2_Wk"][:kw, c, 128 * g:128 * g + 128],
                            rhs=ctxT_sb[:kw, c, :],
                            start=(c == 0), stop=(c == len(CTX_CHUNKS) - 1))
                    nc.vector.tensor_copy(out=K2f[:, g, :], in_=ps[:, :MCTX])
                ps = ps_acc([128, 512], name="v2_ps")
                for c, (k0, kw) in enumerate(CTX_CHUNKS):
                    nc.tensor.matmul(
                        ps[:MCTX, :DIM], lhsT=ctxT_sb[:kw, c, :],
                        rhs=wsb["a2_Wv"][:kw, c, :],
                        start=(c == 0), stop=(c == len(CTX_CHUNKS) - 1))
                nc.vector.tensor_copy(
                    out=V2r[:MCTX, 0:328].rearrange("p (h c) -> p h c", c=41)[:, :, 0:40],
                    in_=ps[:MCTX, :DIM].rearrange("p (h c) -> p h c", c=40))
                nc.vector.memset(
                    V2r[:MCTX, 0:328].rearrange("p (h c) -> p h c",
                                                c=41)[:, :, 40:41], 1.0)

            # ---- own rows first: LN1 -> hqT, Qf (so attention can start as
            # soon as the leading K/V blocks exist; xq was DMA'd first above)
            for qb in range(QT // 4):
                hb = work.tile([128, 4, DIMP], bf, tag="hblk", bufs=2, name="hq")
                for tt in range(4):
                    t = qb * 4 + tt
                    ln_into(hb[:, tt, :], resid[:, t, :], "ln1_w", "ln1_b")
                transpose_blk(actT[:, qb * 4:qb * 4 + 4, :, :], hb,
                              "scalar" if qb % 2 == 0 else "sync")
            proj_fm(QfA, wsb["a1_WqA"], actT, 0, QT, DIM_CHUNKS)
            proj_fm(QfB, wsb["a1_WqB"], actT, 0, QT, DIM_CHUNKS)

            # ---- attn1 building blocks
            def attn1_scores_exp(q0, hp, kb):
                sc = ps_sc([128, 1024], name="sc")
                # full-128 contraction with ONE shared stationary (K group
                # hp): Qf{A,B} hold only the even/odd head at its partition
                # offset with exact zeros elsewhere, so the other head's K
                # rows contribute nothing. Keeps all of attn1 in (128,128)
                # tile mode (no PE mode-switch drains) and halves score LDWs.
                kblk = Kf[:, hp, kb * 128:(kb + 1) * 128]
                nc.tensor.matmul(sc[:, 0:512], lhsT=kblk,
                                 rhs=QfA[:, hp, q0:q0 + 512],
                                 start=True, stop=True)
                nc.tensor.matmul(sc[:, 512:1024], lhsT=kblk,
                                 rhs=QfB[:, hp, q0:q0 + 512],
                                 start=True, stop=True)
                ep = expp.tile([128, 1024], f8, tag="ep", name="ep")
                if KB >= 8 and kb % 8 in (1, 3, 4, 6):
                    # exp(z) ~ (1+z/32)^32 on the vector engine (softmax-
                    # invariant constant error) to offload ACT
                    nc.vector._custom_dve(exp_op, out=ep, in0=sc,
                                          s0=SCALE / 32.0, s1=1.0)
                else:
                    nc.scalar.activation(ep, sc, AF.Exp, scale=SCALE)
                return ep

            def attn1_pv(acc, hp, kb, ep):
                for j in range(2):
                    hh = 2 * hp + j
                    for qs in range(4):
                        nc.tensor.matmul(
                            acc[qs][:, 41 * hh:41 * hh + 41],
                            lhsT=ep[:, j * 512 + qs * 128:j * 512 + (qs + 1) * 128],
                            rhs=Vr[:, kb, 41 * hh:41 * hh + 41],
                            start=(kb == 0), stop=(kb == KB - 1),
                            skip_group_check=True)

            # ---- LN1 + K/V production, merged with attn1 half-0.
            # Block nb's LN/transpose/K/V-proj items are interleaved ~1:1.2
            # with block (nb-1)'s score items, so the exp latency of half-0
            # hides under proj PE work (and vice versa). Scores and K/V proj
            # share the "sc" PSUM tag; the half-0 PV accumulators hold the
            # "acc" tag throughout.
            xts = {}

            def load_xt(t):
                xt = work.tile([128, DIMP], bf, tag="xt", bufs=5, name="xt")
                nc.sync.dma_start(out=xt, in_=xfull_d[t * 128:(t + 1) * 128, :])
                xts[t] = xt

            def kproj_item(nb, g):
                ps = ps_acc([128, 512], name="kf_ps")
                for c, (k0, kw) in enumerate(DIM_CHUNKS):
                    nc.tensor.matmul(
                        ps,
                        lhsT=wsb["a1_Wk"][:128, c, 128 * g:128 * g + 128],
                        rhs=h1T[:128, nb * 4:(nb + 1) * 4, c, :],
                        start=(c == 0), stop=(c == len(DIM_CHUNKS) - 1))
                nc.scalar.activation(Kf[:, g, nb * 512:(nb + 1) * 512], ps,
                                     AF.Identity)

            def vproj_item(t):
                ps = ps_acc([128, 512], name="v_ps")
                for c, (k0, kw) in enumerate(DIM_CHUNKS):
                    nc.tensor.matmul(
                        ps[:, :DIM],
                        lhsT=h1T[:128, t, c, :],
                        rhs=wsb["a1_Wv"][:128, c, :],
                        start=(c == 0), stop=(c == len(DIM_CHUNKS) - 1))
                nc.vector.tensor_copy(
                    out=Vr[:, t, 0:328].rearrange("p (h c) -> p h c",
                                                  c=41)[:, :, 0:40],
                    in_=ps[:, :DIM].rearrange("p (h c) -> p h c", c=40))

            # ones-denominator columns for ALL kb blocks, written once before
            # any V data or PV read (vproj writes never touch column 40)
            nc.vector.memset(
                Vr[:, 0:KB, 0:328].rearrange(
                    "p b (h c) -> p b h c", c=41)[:, :, :, 40], 1.0)
            for t in range(4):
                load_xt(t)

            # ---- shared attention epilogue: normalize, transpose, proj, add
            def finish_attn_norm(acc):
                """Normalize the PV accumulators into an SBUF block and issue
                its transpose; reads (and thus frees) the acc PSUM banks."""
                ab = work.tile([128, 4, DIMP], bf, tag="armblk", bufs=1,
                               name="armblk")
                nc.vector.memset(ab[:, :, DIM:DIMP], 0.0)
                for qs in range(4):
                    rec = work.tile([128, HEADS], f32, tag="rec", name="rec")
                    nc.vector.reciprocal(
                        rec, acc[qs].rearrange("p (h c) -> p h c", c=41)[:, :, 40])
                    rb = bass.AP(tensor=rec.tensor, offset=rec.offset,
                                 ap=[list(rec.ap[0]), [rec.ap[1][0], HEADS],
                                     [0, 40]])
                    nc.vector.tensor_mul(
                        out=ab[:, qs, 0:DIM].rearrange("p (h c) -> p h c", c=40),
                        in0=acc[qs].rearrange("p (h c) -> p h c", c=41)[:, :, 0:40],
                        in1=rb)
                afm = work.tile([128, 4, 3, 128], bf, tag="afm", bufs=2,
                                name="afm")
                transpose_blk(afm, ab)
                return afm

            def finish_attn_proj(qh, afm, wo, bo_key):
                for qs in range(4):
                    po = ps_acc([128, DIM], name="po")
                    for c, (k0, kw) in enumerate(DIM_CHUNKS):
                        nc.tensor.matmul(po, lhsT=afm[:128, qs, c, :],
                                         rhs=wo[:128, c, :],
                                         start=(c == 0),
                                         stop=(c == len(DIM_CHUNKS) - 1))
                    t = qh * 4 + qs
                    nc.vector.tensor_add(out=resid[:, t, 0:DIM],
                                         in0=resid[:, t, 0:DIM], in1=po)
                    if bo_key in flags:
                        nc.vector.tensor_add(out=resid[:, t, 0:DIM],
                                             in0=resid[:, t, 0:DIM],
                                             in1=bcast[bo_key])

            NMT = (2 * IFF) // 128  # 20

            def attn1_half(qh):
                """Self-attention scores+PV for one q-half (PV pipelined two
                tiles back). Returns the acc PSUM tiles."""
                q0 = qh * 512
                acc = [ps_acc([128, HEADS * 41], name=f"acc{qs}")
                       for qs in range(4)]
                pending = []
                for hp in range(HEADS // 2):
                    for kb in range(KB):
                        ep = attn1_scores_exp(q0, hp, kb)
                        pending.append((hp, kb, ep))
                        if kb % 4 == 3:
                            while len(pending) > 2:
                                attn1_pv(acc, *pending.pop(0))
                for phk in pending:
                    attn1_pv(acc, *phk)
                return acc

            def attn1_half0_merged():
                """KV production for block nb interleaved with block (nb-1)'s
                half-0 score/exp/PV items."""
                acc = [ps_acc([128, HEADS * 41], name=f"acc{qs}")
                       for qs in range(4)]
                pending = []

                def score_item(hp, kb):
                    ep = attn1_scores_exp(0, hp, kb)
                    pending.append((hp, kb, ep))
                    if len(pending) > 2:
                        attn1_pv(acc, *pending.pop(0))

                NB = S // 512
                for nb in range(NB + 1):
                    thunks = []
                    if nb < NB:
                        hb = work.tile([128, 4, DIMP], bf, tag="hblk", bufs=2,
                                       name="h1")
                        for tt in range(4):
                            t = nb * 4 + tt
                            if t + 4 < S // 128:
                                load_xt(t + 4)
                            thunks.append((ln_into, hb[:, tt, :], xts.pop(t),
                                           "ln1_w", "ln1_b"))
                        thunks.append((transpose_blk,
                                       h1T[:, nb * 4:nb * 4 + 4, :, :], hb,
                                       "scalar" if nb % 2 == 0 else "sync"))
                        for g in range(4):
                            thunks.append((kproj_item, nb, g))
                        for tt in range(4):
                            thunks.append((vproj_item, nb * 4 + tt))
                    sitems = []
                    if nb > 0:
                        for kb in range((nb - 1) * 4, nb * 4):
                            for hp in range(4):
                                sitems.append((hp, kb))
                    si = 0
                    for i, th in enumerate(thunks):
                        th[0](*th[1:])
                        tgt = (len(sitems) * (i + 1)) // max(len(thunks), 1)
                        while si < min(tgt, len(sitems)):
                            score_item(*sitems[si])
                            si += 1
                    while si < len(sitems):
                        score_item(*sitems[si])
                        si += 1
                while pending:
                    attn1_pv(acc, *pending.pop(0))
                return acc

            def ln_block(qh, wkey, bkey):
                hb = work.tile([128, 4, DIMP], bf, tag="hblk", bufs=2, name="hb")
                for tt in range(4):
                    t = qh * 4 + tt
                    ln_into(hb[:, tt, :], resid[:, t, :], wkey, bkey)
                transpose_blk(actT[:, qh * 4:qh * 4 + 4, :, :], hb)

            def attn2_half(qh):
                q0 = qh * 512
                proj_fm(Q2fA, wsb["a2_WqA"], actT, qh * 4, qh * 4 + 4,
                        DIM_CHUNKS, tbase=qh * 4)
                proj_fm(Q2fB, wsb["a2_WqB"], actT, qh * 4, qh * 4 + 4,
                        DIM_CHUNKS, tbase=qh * 4)
                acc = [ps_acc([128, HEADS * 41], name=f"acc2_{qs}")
                       for qs in range(4)]
                p2 = []
                for hp in range(HEADS // 2):
                    sc = ps_sc([128, 1024], name="sc2")
                    k2blk = K2f[:, hp, :]
                    nc.tensor.matmul(sc[:MCTX, 0:512], lhsT=k2blk,
                                     rhs=Q2fA[:, hp, 0:512],
                                     start=True, stop=True)
                    nc.tensor.matmul(sc[:MCTX, 512:1024], lhsT=k2blk,
                                     rhs=Q2fB[:, hp, 0:512],
                                     start=True, stop=True)
                    ep = expp.tile([128, 1024], f8, tag="ep2", bufs=2, name="ep2")
                    if hp % 2 == 1:
                        nc.vector._custom_dve(exp_op, out=ep[:MCTX, :],
                                              in0=sc[:MCTX, :],
                                              s0=SCALE / 32.0, s1=1.0)
                    else:
                        nc.scalar.activation(ep[:MCTX, :], sc[:MCTX, :], AF.Exp,
                                             scale=SCALE)
                    p2.append((hp, ep))
                for hp, ep in p2:
                    for j in range(2):
                        hh = 2 * hp + j
                        for qs in range(4):
                            nc.tensor.matmul(
                                acc[qs][:, 41 * hh:41 * hh + 41],
                                lhsT=ep[:MCTX, j * 512 + qs * 128:
                                        j * 512 + (qs + 1) * 128],
                                rhs=V2r[:MCTX, 41 * hh:41 * hh + 41],
                                start=True, stop=True, skip_group_check=True)
                return acc

            def ff_inner(qh):
                _order = [m for pair in zip(range(NMT // 2), range(NMT // 2, NMT))
                          for m in pair]
                for mt in _order:
                    ps = ps_acc([128, 512], name="ff1_ps")
                    for c, (k0, kw) in enumerate(DIM_CHUNKS):
                        nc.tensor.matmul(
                            ps, lhsT=wsb["ff_W1"][:128, c, mt * 128:(mt + 1) * 128],
                            rhs=actT[:128, qh * 4:qh * 4 + 4, c, :],
                            start=(c == 0), stop=(c == len(DIM_CHUNKS) - 1))
                    if mt < NMT // 2:
                        nc.scalar.activation(Uff[:, mt, :], ps,
                                             AF.Identity,
                                             bias=b1t[:, mt:mt + 1], scale=1.0)
                    else:
                        gl = work.tile([128, 512], bf, tag="gel", name="gel")
                        nc.scalar.activation(gl, ps, AF.Gelu,
                                             bias=b1t[:, mt:mt + 1], scale=1.0)
                        mu = mt - NMT // 2
                        nc.vector.tensor_mul(out=Uff[:, mu, :],
                                             in0=Uff[:, mu, :], in1=gl)

            def ff_out(qh):
                for tt in range(4):
                    qs = qh * 4 + tt
                    po = ps_acc([128, DIM], name="ff2_ps")
                    for c in range(IFF // 128):
                        nc.tensor.matmul(po,
                                         lhsT=Uff[:, c, tt * 128:(tt + 1) * 128],
                                         rhs=w2_sb[:, c, :],
                                         start=(c == 0), stop=(c == IFF // 128 - 1))
                    ot = work.tile([128, DIM], f32, tag="ot", name="ot")
                    nc.vector.tensor_add(out=ot, in0=resid[:, qs, 0:DIM], in1=po)
                    if "ff_b2" in flags:
                        nc.vector.tensor_add(out=ot, in0=ot, in1=bcast["ff_b2"])
                    nc.sync.dma_start(out=out_d[qs * 128:(qs + 1) * 128, :], in_=ot)

            # ============ emission schedule: the two q-halves are interleaved
            # within each phase so every finish/LN transpose's DMA latency is
            # covered by the other half's matmul stream.
            NB = S // 512
            for nb in range(NB):
                hb = work.tile([128, 4, DIMP], bf, tag="hblk", bufs=2, name="h1")
                for tt in range(4):
                    t = nb * 4 + tt
                    if t + 4 < S // 128:
                        load_xt(t + 4)
                    ln_into(hb[:, tt, :], xts.pop(t), "ln1_w", "ln1_b")
                transpose_blk(h1T[:, nb * 4:nb * 4 + 4, :, :], hb,
                              "scalar" if nb % 2 == 0 else "sync")
                for g in range(4):
                    kproj_item(nb, g)
                for tt in range(4):
                    vproj_item(nb * 4 + tt)
            acc = attn1_half(0)
            afm0 = finish_attn_norm(acc)          # frees acc banks for qh1
            # cross-attn/FF weights + context K,V hide under attn1(qh0)/(qh1)
            w2_sb, b1t, ctxT_sb = load_late_weights()
            cross_kv()
            acc = attn1_half(1)
            afm1 = finish_attn_norm(acc)
            finish_attn_proj(0, afm0, wsb["a1_Wo"], "a1_bo")
            ln_block(0, "ln2_w", "ln2_b")
            finish_attn_proj(1, afm1, wsb["a1_Wo"], "a1_bo")
            acc = attn2_half(0)
            afm0 = finish_attn_norm(acc)
            ln_block(1, "ln2_w", "ln2_b")
            finish_attn_proj(0, afm0, wsb["a2_Wo"], "a2_bo")
            acc = attn2_half(1)
            afm1 = finish_attn_norm(acc)
            ln_block(0, "ln3_w", "ln3_b")
            finish_attn_proj(1, afm1, wsb["a2_Wo"], "a2_bo")
            ff_inner(0)
            ln_block(1, "ln3_w", "ln3_b")
            ff_out(0)
            ff_inner(1)
            ff_out(1)

    nc.compile()
    return nc


_CACHE = {}


def _get_nc(S, R, flags):
    key = (S, R, tuple(sorted(flags)))
    if key not in _CACHE:
        _CACHE[key] = build_nc(S, R, flags)
    return _CACHE[key]


def _pad_qk8(w):
    """Self-attn Q/K weight layout for fp8 DoubleRow: per head h (g=h//4,
    m=h%4), sub i: block col 128*(2g+i) + 32*m + dk <- w col 40h + 20i + dk."""
    w = np.asarray(w)
    out = np.zeros((w.shape[0], 512), w.dtype)
    for h in range(HEADS):
        g, m = divmod(h, 4)
        for i in range(2):
            c0 = 128 * (2 * g + i) + 32 * m
            out[:, c0:c0 + 20] = w[:, DH * h + 20 * i:DH * h + 20 * i + 20]
    return out


def _pad_qk2(w, par=None):
    """Q/K layout: 2-head groups at partition offsets {0,64}. With par set,
    only even (par=0) or odd (par=1) heads are kept (others zero) so the
    score matmul can contract over all 128 partitions with one shared K."""
    w = np.asarray(w)
    out = np.zeros((w.shape[0], 512), w.dtype)
    for h in range(HEADS):
        g, j = divmod(h, 2)
        if par is not None and j != par:
            continue
        out[:, 128 * g + 64 * j:128 * g + 64 * j + DH] = w[:, DH * h:DH * h + DH]
    return out


def make_in_maps(x, context, ln_params, weights):
    """Host-side prep: returns (flags, in_maps, R, S, Bn)."""
    x = np.asarray(x)
    context = np.asarray(context)
    Bn = x.shape[0]
    S = x.shape[1]
    R = S * Bn // NCORES
    flags = set()
    for nm in ("ln1_w", "ln2_w", "ln3_w"):
        if not np.allclose(np.asarray(ln_params[nm]), 1.0):
            flags.add(nm)
    for nm in ("ln1_b", "ln2_b", "ln3_b", "a1_bo", "a2_bo", "ff_b2"):
        if not np.allclose(np.asarray(ln_params[nm]), 0.0):
            flags.add(nm)
    weights = dict(weights)
    w1q = weights.pop("a1_Wq")
    weights["a1_WqA"] = _pad_qk2(w1q, 0)
    weights["a1_WqB"] = _pad_qk2(w1q, 1)
    weights["a1_Wk"] = _pad_qk2(weights["a1_Wk"])
    w2q = weights.pop("a2_Wq")
    weights["a2_WqA"] = _pad_qk2(w2q, 0)
    weights["a2_WqB"] = _pad_qk2(w2q, 1)
    weights["a2_Wk"] = _pad_qk2(weights["a2_Wk"])
    shared = {nm: np.ascontiguousarray(np.asarray(w).astype(BF16))
              for nm, w in weights.items()}
    shared["ff_b1"] = np.ascontiguousarray(
        np.asarray(ln_params["ff_b1"]).astype(np.float32))
    for nm in flags:
        shared[nm] = np.ascontiguousarray(
            np.asarray(ln_params[nm]).astype(np.float32))
    pad = ((0, 0), (0, 0), (0, DIMP - DIM))
    xbf = np.ascontiguousarray(np.pad(x, pad).astype(BF16))
    ctxT = np.ascontiguousarray(np.asarray(context).astype(BF16).transpose(0, 2, 1))
    xf32 = np.ascontiguousarray(np.pad(x, pad).astype(np.float32))
    in_maps = []
    cpb = NCORES // Bn
    for core in range(NCORES):
        b, c = divmod(core, cpb)
        m = dict(shared)
        m["xfull"] = xbf[b]
        m["xq"] = np.ascontiguousarray(xf32[b, c * R:(c + 1) * R])
        m["ctxT"] = ctxT[b]
        in_maps.append(m)
    return flags, in_maps, R, S, Bn


def kernel(x, context, ln1_w, ln1_b, ln2_w, ln2_b, ln3_w, ln3_b,
           a1_Wq, a1_Wk, a1_Wv, a1_Wo, a1_bo,
           a2_Wq, a2_Wk, a2_Wv, a2_Wo, a2_bo,
           ff_W1, ff_b1, ff_W2, ff_b2, _trace=False):
    from concourse.bass_utils import run_bass_kernel_spmd

    weights = dict(a1_Wq=a1_Wq, a1_Wk=a1_Wk, a1_Wv=a1_Wv, a1_Wo=a1_Wo,
                   a2_Wq=a2_Wq, a2_Wk=a2_Wk, a2_Wv=a2_Wv, a2_Wo=a2_Wo,
                   ff_W1=ff_W1, ff_W2=ff_W2)
    ln_params = dict(ln1_w=ln1_w, ln1_b=ln1_b, ln2_w=ln2_w, ln2_b=ln2_b,
                     ln3_w=ln3_w, ln3_b=ln3_b, a1_bo=a1_bo, a2_bo=a2_bo,
                     ff_b1=ff_b1, ff_b2=ff_b2)
    flags, in_maps, R, S, Bn = make_in_maps(x, context, ln_params, weights)
    nc = _get_nc(S, R, flags)
    res = run_bass_kernel_spmd(nc, in_maps, core_ids=list(range(NCORES)),
                               trace=_trace)
    out = np.empty((Bn, S, DIM), np.float32)
    cpb = NCORES // Bn
    for core in range(NCORES):
        b, c = divmod(core, cpb)
        out[b, c * R:(c + 1) * R] = res.results[core]["out"]
    kernel._last_result = res
    return out



# revision 45
# speedup vs baseline: 1.2246x; 1.1689x over previous
"""Trainium2 Bass kernel for a BasicTransformerBlock (self-attn + cross-attn + GEGLU FF).

Sharding: 8 cores = 2 batches x 4 sequence chunks of 1024 rows. Each core
redundantly computes LN1 + K/V projections over its batch's full 4096 rows
(position-independent, so all cores run an identical SPMD program) and
produces its own 1024-row slice of the output. No collectives.

Precision: fp32 residual stream and softmax statistics; bf16 weights and
activations for projections/FF; fp8e3m4 for the softmax probabilities and V
in the probability-x-V matmuls (errors there are diluted ~100x by the fp32
residual). Softmax runs without max-subtraction (scores are provably small
at this problem's scale: |s| < ~1.1) with 1/sqrt(dh) folded into the exp;
the denominator comes free from a ones-column in V.

All row-major -> feature-major transposes run on the DMA xbar engines
(single [128, 4x384] block transposes, issued from the ACT queue in the
LN1/KV phase and the SP queue elsewhere), keeping the PE array free for
matmuls; the softmax exp is split ~50/50 between ACT (hardware Exp) and the
vector engine (custom (1+z/32)^32 DVE op). The two q-halves are interleaved
per phase so attention-epilogue transpose latency hides under the other
half's matmul stream.
"""

import numpy as np
import ml_dtypes

DIM = 320
DIMP = 384  # DIM padded to a multiple of 128 for DMA-xbar transposes
HEADS = 8
DH = 40
CTX = 768
IFF = 1280  # GEGLU inner width; proj1 width = 2*IFF
EPS = 1e-5
SCALE = DH ** -0.5
NCORES = 8
MCTX = 77
VS = 336  # V row stride (8*41 = 328 padded to %16 for DoubleRow)

BF16 = ml_dtypes.bfloat16


def _chunks(total, step=128):
    out = []
    k = 0
    while k < total:
        out.append((k, min(step, total - k)))
        k += step
    return out


DIM_CHUNKS = _chunks(DIM)    # [(0,128),(128,128),(256,64)]
CTX_CHUNKS = _chunks(CTX)    # 6 x 128


def _register_exp_op():
    """Custom DVE op: out = (in0*s0 + s1)^32 — used as exp(z) ~ (1+z/32)^32
    to offload part of the softmax exp from ACT to the vector engine."""
    import concourse.dve_ops as dve_ops
    for o in dve_ops.OPS:
        if o.name == "EXP_POLY32_ANT":
            return o
    from concourse.dve_spec import Spec, Src0, C0, C1, sq
    spec = Spec(
        body=sq(sq(sq(sq(sq(Src0 * C0 + C1))))),
        reference=lambda in0, in1, s0, s1, imm2:
            ((in0.astype(np.float32) * s0 + s1) ** 32).astype(np.float32))
    op = dve_ops.DveOp("EXP_POLY32_ANT", spec, subdim=False,
                       uops_sha={"v3": "eafb894a1d5c531b"})
    dve_ops.OPS.append(op)
    dve_ops._SUB_OPCODE_FOR_NAME[op.name] = \
        dve_ops._CUSTOM_DVE_ROW_BASE + len(dve_ops.OPS) - 1
    dve_ops.CUSTOM_DVE_SPECS[op.name] = op.spec
    return op


def build_nc(S, R, flags=()):
    """Build + compile the per-core Bass program.

    flags: subset of {"ln1_w","ln1_b","ln2_w","ln2_b","ln3_w","ln3_b",
    "a1_bo","a2_bo","ff_b2"} that are non-trivial and must be applied.
    """
    import concourse.bass as bass
    import concourse.tile as tile
    from concourse import bacc, mybir
    from concourse.masks import make_identity

    f32 = mybir.dt.float32
    bf = mybir.dt.bfloat16
    f8 = mybir.dt.float8e3
    AF = mybir.ActivationFunctionType
    OP = mybir.AluOpType
    PM = mybir.MatmulPerfMode
    flags = set(flags)

    KB = S // 128     # key blocks (self-attn)
    QT = R // 128     # q row-tiles
    QHS = R // 512    # q 512-row groups

    nc = bacc.Bacc("TRN2", target_bir_lowering=False, debug=False)

    def din(name, shape, dt=bf):
        return nc.dram_tensor(name, shape, dt, kind="ExternalInput").ap()

    xfull_d = din("xfull", [S, DIMP])
    xq_d = din("xq", [R, DIMP], f32)
    ctxT_d = din("ctxT", [CTX, MCTX])
    w_d = {}
    for nm, shape in [
        ("a1_WqA", [DIM, 512]), ("a1_WqB", [DIM, 512]),
        ("a2_WqA", [DIM, 512]), ("a2_WqB", [DIM, 512]),
        ("a1_Wk", [DIM, 512]), ("a1_Wv", [DIM, DIM]),
        ("a1_Wo", [DIM, DIM]), ("a2_Wk", [CTX, 512]),
        ("a2_Wv", [CTX, DIM]), ("a2_Wo", [DIM, DIM]),
        ("ff_W1", [DIM, 2 * IFF]), ("ff_W2", [IFF, DIM]),
    ]:
        w_d[nm] = din(nm, shape)
    b1_d = din("ff_b1", [2 * IFF], f32)
    vec_d = {nm: din(nm, [DIM], f32) for nm in sorted(flags)}
    out_d = nc.dram_tensor("out", [R, DIM], f32, kind="ExternalOutput").ap()

    with tile.TileContext(nc) as tc:
        import contextlib
        with contextlib.ExitStack() as est:
            persist = est.enter_context(tc.tile_pool(name="persist", bufs=1))
            work = est.enter_context(tc.tile_pool(name="work", bufs=4))
            expp = est.enter_context(tc.tile_pool(name="expp", bufs=5))
            # One PSUM pool for the whole kernel: tag "sc" = 2 x [128,1024]f32
            # (4 banks), tag "acc" = 4 x [128,512]f32 (4 banks). All other
            # PSUM tiles allocate from these tags so phases can pipeline.
            psum = est.enter_context(tc.tile_pool(name="psum", bufs=2,
                                                  space="PSUM"))

            def ps_sc(shape, dt=f32, name="sc"):
                return psum.tile(shape, dt, tag="sc", bufs=4, name=name)

            def ps_acc(shape, dt=f32, name="accp"):
                return psum.tile(shape, dt, tag="acc", bufs=4, name=name)

            ident = persist.tile([128, 128], bf, name="ident")
            make_identity(nc, ident)
            eps_t = persist.tile([128, 1], f32, name="eps_t")
            nc.vector.memset(eps_t, EPS)

            # ---- persistent activations
            # h1T/actT are t-block-major [128, T, 3, 128]: each [p, t, :, :]
            # slice is per-partition contiguous (3*128), the layout the DMA
            # xbar transpose requires for its destination.
            h1T = persist.tile([128, S // 128, 3, 128], bf, name="h1T")
            Kf = persist.tile([128, 4, S], bf, name="Kf")        # 2-head blocks
            QfA = persist.tile([128, 4, R], bf, name="QfA")
            QfB = persist.tile([128, 4, R], bf, name="QfB")
            Vr = persist.tile([128, KB, VS], f8, name="Vr")
            K2f = persist.tile([128, 4, MCTX], bf, name="K2f")   # 2-head blocks
            # one q-half at a time (halves are processed sequentially)
            Q2fA = persist.tile([128, 4, 512], bf, name="Q2fA")
            Q2fB = persist.tile([128, 4, 512], bf, name="Q2fB")
            V2r = persist.tile([128, VS], f8, name="V2r")
            actT = persist.tile([128, QT, 3, 128], bf, name="actT")  # hqT/h2T/h3T
            resid = persist.tile([128, QT, DIMP], f32, name="resid")
            # Uff holds one q-half at a time (ff_out(qh) drains it before
            # ff_inner(qh+1) refills; PE program order enforces this anyway)
            Uff = persist.tile([128, IFF // 128, 512], bf, name="Uff")


            # ---- weights into SBUF, [in, out] layout chunked on partitions.
            # Loaded in stages so the critical path (xq/xfull -> LN1 -> Q/K/V)
            # is not queued behind 4.7MB of cross-attn/FF weights.
            wsb = {}

            def load_w(names):
                for nm in names:
                    chks = CTX_CHUNKS if nm in ("a2_Wk", "a2_Wv") else DIM_CHUNKS
                    width = w_d[nm].shape[1]
                    t = persist.tile([128, len(chks), width], bf, name=f"w_{nm}",
                                     uniquify=True)
                    for c, (k0, kw) in enumerate(chks):
                        nc.sync.dma_start(out=t[:kw, c, :],
                                          in_=w_d[nm][k0:k0 + kw, :])
                        if kw < 128:
                            # zero-pad so every chunk matmul runs with a full
                            # 128 contraction (uniform (128,128) tile mode —
                            # avoids PE mode-switch drains; extra rows hit
                            # zero weights and contraction depth is free)
                            nc.vector.memset(t[kw:128, c, :], 0.0)
                    wsb[nm] = t

            for t in range(QT):
                nc.sync.dma_start(out=resid[:, t, :],
                                  in_=xq_d[t * 128:(t + 1) * 128, :])
            load_w(["a1_WqA", "a1_WqB", "a1_Wk", "a1_Wv"])

            bcast = {}
            for nm in sorted(flags):
                t = persist.tile([128, DIM], f32, name=f"bc_{nm}")
                src = vec_d[nm]
                bc_ap = bass.AP(tensor=src.tensor, offset=src.offset,
                                ap=[[0, 128]] + [list(p) for p in src.ap])
                nc.gpsimd.dma_start(out=t, in_=bc_ap)
                bcast[nm] = t

            def ln_into(dst_bf, src_ap, wkey, bkey):
                stats = work.tile([128, 6], f32, tag="bnst", name="stats")
                nc.vector.bn_stats(stats, src_ap[:, 0:DIM])
                mv = work.tile([128, 2], f32, tag="bnagg", name="mv")
                nc.vector.bn_aggr(mv, stats)
                rstd = work.tile([128, 1], f32, tag="rstd", name="rstd")
                nc.scalar.activation(rstd, mv[:, 1:2], AF.Sqrt, bias=eps_t, scale=1.0)
                nc.vector.reciprocal(rstd, rstd)
                nc.vector.tensor_scalar(
                    out=dst_bf, in0=src_ap, scalar1=mv[:, 0:1], scalar2=rstd,
                    op0=OP.subtract, op1=OP.mult)
                if wkey in flags:
                    nc.vector.tensor_mul(out=dst_bf[:, 0:DIM],
                                         in0=dst_bf[:, 0:DIM], in1=bcast[wkey])
                if bkey in flags:
                    nc.vector.tensor_add(out=dst_bf[:, 0:DIM],
                                         in0=dst_bf[:, 0:DIM], in1=bcast[bkey])

            def transpose_blk(dstT4, src_blk, engine="sync"):
                """DMA-xbar transpose of a 4-tile [128, 4, DIMP] bf16 block
                into four per-partition-contiguous [128, 3, 128] feature-major
                blocks (dstT4 = [128, 4, 3, 128] slice). One instruction per
                512 rows; issued from SP or ACT to spread queue load."""
                eng = nc.sync if engine == "sync" else nc.scalar
                eng.dma_start_transpose(
                    out=dstT4.rearrange("p t d j -> p (t d) j"),
                    in_=src_blk.rearrange("p t f -> p (t f)"))

            def proj_fm(dst, wt, srcT, t_lo, t_hi, chks, copy_engine="dve",
                        tbase=0):
                """Feature-major projection via stationary (padded) weight cols.

                srcT is t-block-major [128, T, 3, 128]; processes rows
                t_lo*128 .. t_hi*128 in 512-wide groups (dst columns offset
                by tbase*128)."""
                for g in range(4):
                    for t0 in range(t_lo, t_hi, 4):
                        n0 = (t0 - tbase) * 128
                        ps = ps_acc([128, 512], name="proj_ps")
                        for c, (k0, kw) in enumerate(chks):
                            nc.tensor.matmul(
                                ps,
                                lhsT=wt[:128, c, 128 * g:128 * g + 128],
                                rhs=srcT[:128, t0:t0 + 4, c, :],
                                start=(c == 0), stop=(c == len(chks) - 1))
                        eng = copy_engine if copy_engine != "mix" else \
                            ("act" if (g + t0 // 4) % 2 == 0 else "dve")
                        if eng == "act":
                            nc.scalar.activation(dst[:, g, n0:n0 + 512], ps,
                                                 AF.Identity)
                        else:
                            nc.vector.tensor_copy(out=dst[:, g, n0:n0 + 512],
                                                  in_=ps)

            exp_op = _register_exp_op()

            def load_late_weights():
                load_w(["a1_Wo", "a2_WqA", "a2_WqB", "a2_Wk", "a2_Wv", "a2_Wo",
                        "ff_W1"])
                w2 = persist.tile([128, IFF // 128, DIM], bf, name="w_ff2")
                for c in range(IFF // 128):
                    nc.sync.dma_start(out=w2[:, c, :],
                                      in_=w_d["ff_W2"][c * 128:(c + 1) * 128, :])
                b1 = persist.tile([128, (2 * IFF) // 128], f32, name="b1t")
                nc.sync.dma_start(out=b1, in_=b1_d.rearrange("(c p) -> p c", p=128))
                ctxm = persist.tile([128, len(CTX_CHUNKS), MCTX], bf, name="ctxT_sb")
                for c, (k0, kw) in enumerate(CTX_CHUNKS):
                    nc.sync.dma_start(out=ctxm[:kw, c, :], in_=ctxT_d[k0:k0 + kw, :])
                return w2, b1, ctxm

            def cross_kv():
                for g in range(4):
                    ps = ps_sc([128, 128], name="k2_ps")
                    for c, (k0, kw) in enumerate(CTX_CHUNKS):
                        nc.tensor.matmul(
                            ps[:, :MCTX],
                            lhsT=wsb["a2_Wk"][:kw, c, 128 * g:128 * g + 128],
                            rhs=ctxT_sb[:kw, c, :],
                            start=(c == 0), stop=(c == len(CTX_CHUNKS) - 1))
                    nc.vector.tensor_copy(out=K2f[:, g, :], in_=ps[:, :MCTX])
                ps = ps_acc([128, 512], name="v2_ps")
                for c, (k0, kw) in enumerate(CTX_CHUNKS):
                    nc.tensor.matmul(
                        ps[:MCTX, :DIM], lhsT=ctxT_sb[:kw, c, :],
                        rhs=wsb["a2_Wv"][:kw, c, :],
                        start=(c == 0), stop=(c == len(CTX_CHUNKS) - 1))
                nc.vector.tensor_copy(
                    out=V2r[:MCTX, 0:328].rearrange("p (h c) -> p h c", c=41)[:, :, 0:40],
                    in_=ps[:MCTX, :DIM].rearrange("p (h c) -> p h c", c=40))
                nc.vector.memset(
                    V2r[:MCTX, 0:328].rearrange("p (h c) -> p h c",
                                                c=41)[:, :, 40:41], 1.0)

            # ---- own rows first: LN1 -> hqT, Qf (so attention can start as
            # soon as the leading K/V blocks exist; xq was DMA'd first above)
            for qb in range(QT // 4):
                hb = work.tile([128, 4, DIMP], bf, tag="hblk", bufs=2, name="hq")
                for tt in range(4):
                    t = qb * 4 + tt
                    ln_into(hb[:, tt, :], resid[:, t, :], "ln1_w", "ln1_b")
                transpose_blk(actT[:, qb * 4:qb * 4 + 4, :, :], hb,
                              "scalar" if qb % 2 == 0 else "sync")
            proj_fm(QfA, wsb["a1_WqA"], actT, 0, QT, DIM_CHUNKS)
            proj_fm(QfB, wsb["a1_WqB"], actT, 0, QT, DIM_CHUNKS)

            # ---- attn1 building blocks
            def attn1_scores_exp(q0, hp, kb, j):
                """One head's scores: full-128 contraction with a stationary
                shared between j=0/1 (QfA/QfB zero the other head's
                partitions). [128,512] sc tiles (1 PSUM bank) give a 4-deep
                rotation, doubling the scores->exp chain depth; exp per head
                alternates ACT/DVE."""
                sc = ps_sc([128, 512], name="sc")
                kblk = Kf[:, hp, kb * 128:(kb + 1) * 128]
                nc.tensor.matmul(sc, lhsT=kblk,
                                 rhs=(QfA if j == 0 else QfB)[:, hp,
                                                              q0:q0 + 512],
                                 start=True, stop=True)
                ep = expp.tile([128, 512], f8, tag="ep", bufs=8, name="ep")
                if (2 * kb + j) % 8 in (1, 3, 4, 6):
                    # exp(z) ~ (1+z/32)^32 on the vector engine (softmax-
                    # invariant constant error) to offload ACT
                    nc.vector._custom_dve(exp_op, out=ep, in0=sc,
                                          s0=SCALE / 32.0, s1=1.0)
                else:
                    nc.scalar.activation(ep, sc, AF.Exp, scale=SCALE)
                return ep

            def attn1_pv(acc, hp, kb, j, ep):
                hh = 2 * hp + j
                for qs in range(4):
                    nc.tensor.matmul(
                        acc[qs][:, 41 * hh:41 * hh + 41],
                        lhsT=ep[:, qs * 128:(qs + 1) * 128],
                        rhs=Vr[:, kb, 41 * hh:41 * hh + 41],
                        start=(kb == 0), stop=(kb == KB - 1),
                        skip_group_check=True)

            # ---- LN1 + K/V production, merged with attn1 half-0.
            # Block nb's LN/transpose/K/V-proj items are interleaved ~1:1.2
            # with block (nb-1)'s score items, so the exp latency of half-0
            # hides under proj PE work (and vice versa). Scores and K/V proj
            # share the "sc" PSUM tag; the half-0 PV accumulators hold the
            # "acc" tag throughout.
            xts = {}

            def load_xt(t):
                xt = work.tile([128, DIMP], bf, tag="xt", bufs=5, name="xt")
                nc.sync.dma_start(out=xt, in_=xfull_d[t * 128:(t + 1) * 128, :])
                xts[t] = xt

            def kproj_item(nb, g):
                ps = ps_acc([128, 512], name="kf_ps")
                for c, (k0, kw) in enumerate(DIM_CHUNKS):
                    nc.tensor.matmul(
                        ps,
                        lhsT=wsb["a1_Wk"][:128, c, 128 * g:128 * g + 128],
                        rhs=h1T[:128, nb * 4:(nb + 1) * 4, c, :],
                        start=(c == 0), stop=(c == len(DIM_CHUNKS) - 1))
                nc.scalar.activation(Kf[:, g, nb * 512:(nb + 1) * 512], ps,
                                     AF.Identity)

            def vproj_item(t):
                ps = ps_acc([128, 512], name="v_ps")
                for c, (k0, kw) in enumerate(DIM_CHUNKS):
                    nc.tensor.matmul(
                        ps[:, :DIM],
                        lhsT=h1T[:128, t, c, :],
                        rhs=wsb["a1_Wv"][:128, c, :],
                        start=(c == 0), stop=(c == len(DIM_CHUNKS) - 1))
                nc.vector.tensor_copy(
                    out=Vr[:, t, 0:328].rearrange("p (h c) -> p h c",
                                                  c=41)[:, :, 0:40],
                    in_=ps[:, :DIM].rearrange("p (h c) -> p h c", c=40))

            # ones-denominator columns for ALL kb blocks, written once before
            # any V data or PV read (vproj writes never touch column 40)
            nc.vector.memset(
                Vr[:, 0:KB, 0:328].rearrange(
                    "p b (h c) -> p b h c", c=41)[:, :, :, 40], 1.0)
            for t in range(4):
                load_xt(t)

            # ---- shared attention epilogue: normalize, transpose, proj, add
            def finish_attn_norm(acc):
                """Normalize the PV accumulators into an SBUF block and issue
                its transpose; reads (and thus frees) the acc PSUM banks."""
                ab = work.tile([128, 4, DIMP], bf, tag="armblk", bufs=1,
                               name="armblk")
                nc.vector.memset(ab[:, :, DIM:DIMP], 0.0)
                for qs in range(4):
                    rec = work.tile([128, HEADS], f32, tag="rec", name="rec")
                    nc.vector.reciprocal(
                        rec, acc[qs].rearrange("p (h c) -> p h c", c=41)[:, :, 40])
                    rb = bass.AP(tensor=rec.tensor, offset=rec.offset,
                                 ap=[list(rec.ap[0]), [rec.ap[1][0], HEADS],
                                     [0, 40]])
                    nc.vector.tensor_mul(
                        out=ab[:, qs, 0:DIM].rearrange("p (h c) -> p h c", c=40),
                        in0=acc[qs].rearrange("p (h c) -> p h c", c=41)[:, :, 0:40],
                        in1=rb)
                afm = work.tile([128, 4, 3, 128], bf, tag="afm", bufs=2,
                                name="afm")
                transpose_blk(afm, ab)
                return afm

            def finish_attn_proj(qh, afm, wo, bo_key):
                for qs in range(4):
                    po = ps_acc([128, DIM], name="po")
                    for c, (k0, kw) in enumerate(DIM_CHUNKS):
                        nc.tensor.matmul(po, lhsT=afm[:128, qs, c, :],
                                         rhs=wo[:128, c, :],
                                         start=(c == 0),
                                         stop=(c == len(DIM_CHUNKS) - 1))
                    t = qh * 4 + qs
                    nc.vector.tensor_add(out=resid[:, t, 0:DIM],
                                         in0=resid[:, t, 0:DIM], in1=po)
                    if bo_key in flags:
                        nc.vector.tensor_add(out=resid[:, t, 0:DIM],
                                             in0=resid[:, t, 0:DIM],
                                             in1=bcast[bo_key])

            NMT = (2 * IFF) // 128  # 20

            def attn1_half(qh):
                """Self-attention scores+PV for one q-half (PV pipelined two
                tiles back). Returns the acc PSUM tiles."""
                q0 = qh * 512
                acc = [ps_acc([128, HEADS * 41], name=f"acc{qs}")
                       for qs in range(4)]
                pending = []
                for hp in range(HEADS // 2):
                    for kb in range(KB):
                        for j in range(2):
                            ep = attn1_scores_exp(q0, hp, kb, j)
                            pending.append((hp, kb, j, ep))
                        if kb % 4 == 3:
                            while len(pending) > 4:
                                attn1_pv(acc, *pending.pop(0))
                for phk in pending:
                    attn1_pv(acc, *phk)
                return acc

            def attn1_half0_merged():
                """KV production for block nb interleaved with block (nb-1)'s
                half-0 score/exp/PV items."""
                acc = [ps_acc([128, HEADS * 41], name=f"acc{qs}")
                       for qs in range(4)]
                pending = []

                def score_item(hp, kb):
                    ep = attn1_scores_exp(0, hp, kb)
                    pending.append((hp, kb, ep))
                    if len(pending) > 2:
                        attn1_pv(acc, *pending.pop(0))

                NB = S // 512
                for nb in range(NB + 1):
                    thunks = []
                    if nb < NB:
                        hb = work.tile([128, 4, DIMP], bf, tag="hblk", bufs=2,
                                       name="h1")
                        for tt in range(4):
                            t = nb * 4 + tt
                            if t + 4 < S // 128:
                                load_xt(t + 4)
                            thunks.append((ln_into, hb[:, tt, :], xts.pop(t),
                                           "ln1_w", "ln1_b"))
                        thunks.append((transpose_blk,
                                       h1T[:, nb * 4:nb * 4 + 4, :, :], hb,
                                       "scalar" if nb % 2 == 0 else "sync"))
                        for g in range(4):
                            thunks.append((kproj_item, nb, g))
                        for tt in range(4):
                            thunks.append((vproj_item, nb * 4 + tt))
                    sitems = []
                    if nb > 0:
                        for kb in range((nb - 1) * 4, nb * 4):
                            for hp in range(4):
                                sitems.append((hp, kb))
                    si = 0
                    for i, th in enumerate(thunks):
                        th[0](*th[1:])
                        tgt = (len(sitems) * (i + 1)) // max(len(thunks), 1)
                        while si < min(tgt, len(sitems)):
                            score_item(*sitems[si])
                            si += 1
                    while si < len(sitems):
                        score_item(*sitems[si])
                        si += 1
                while pending:
                    attn1_pv(acc, *pending.pop(0))
                return acc

            def ln_block(qh, wkey, bkey):
                hb = work.tile([128, 4, DIMP], bf, tag="hblk", bufs=2, name="hb")
                for tt in range(4):
                    t = qh * 4 + tt
                    ln_into(hb[:, tt, :], resid[:, t, :], wkey, bkey)
                transpose_blk(actT[:, qh * 4:qh * 4 + 4, :, :], hb)

            def attn2_half(qh):
                q0 = qh * 512
                proj_fm(Q2fA, wsb["a2_WqA"], actT, qh * 4, qh * 4 + 4,
                        DIM_CHUNKS, tbase=qh * 4)
                proj_fm(Q2fB, wsb["a2_WqB"], actT, qh * 4, qh * 4 + 4,
                        DIM_CHUNKS, tbase=qh * 4)
                acc = [ps_acc([128, HEADS * 41], name=f"acc2_{qs}")
                       for qs in range(4)]
                p2 = []
                for hp in range(HEADS // 2):
                    for j in range(2):
                        sc = ps_sc([128, 512], name="sc2")
                        nc.tensor.matmul(sc[:MCTX, :], lhsT=K2f[:, hp, :],
                                         rhs=(Q2fA if j == 0 else
                                              Q2fB)[:, hp, 0:512],
                                         start=True, stop=True)
                        ep = expp.tile([128, 512], f8, tag="ep2", bufs=6,
                                       name="ep2")
                        if (2 * hp + j) % 2 == 1:
                            nc.vector._custom_dve(exp_op, out=ep[:MCTX, :],
                                                  in0=sc[:MCTX, :],
                                                  s0=SCALE / 32.0, s1=1.0)
                        else:
                            nc.scalar.activation(ep[:MCTX, :], sc[:MCTX, :],
                                                 AF.Exp, scale=SCALE)
                        p2.append((2 * hp + j, ep))
                for hh, ep in p2:
                    for qs in range(4):
                        nc.tensor.matmul(
                            acc[qs][:, 41 * hh:41 * hh + 41],
                            lhsT=ep[:MCTX, qs * 128:(qs + 1) * 128],
                            rhs=V2r[:MCTX, 41 * hh:41 * hh + 41],
                            start=True, stop=True, skip_group_check=True)
                return acc

            def ff_inner(qh):
                _order = [m for pair in zip(range(NMT // 2), range(NMT // 2, NMT))
                          for m in pair]
                for mt in _order:
                    ps = ps_acc([128, 512], name="ff1_ps")
                    for c, (k0, kw) in enumerate(DIM_CHUNKS):
                        nc.tensor.matmul(
                            ps, lhsT=wsb["ff_W1"][:128, c, mt * 128:(mt + 1) * 128],
                            rhs=actT[:128, qh * 4:qh * 4 + 4, c, :],
                            start=(c == 0), stop=(c == len(DIM_CHUNKS) - 1))
                    if mt < NMT // 2:
                        nc.scalar.activation(Uff[:, mt, :], ps,
                                             AF.Identity,
                                             bias=b1t[:, mt:mt + 1], scale=1.0)
                    else:
                        gl = work.tile([128, 512], bf, tag="gel", name="gel")
                        nc.scalar.activation(gl, ps, AF.Gelu,
                                             bias=b1t[:, mt:mt + 1], scale=1.0)
                        mu = mt - NMT // 2
                        nc.vector.tensor_mul(out=Uff[:, mu, :],
                                             in0=Uff[:, mu, :], in1=gl)

            def ff_out(qh):
                for tt in range(4):
                    qs = qh * 4 + tt
                    po = ps_acc([128, DIM], name="ff2_ps")
                    for c in range(IFF // 128):
                        nc.tensor.matmul(po,
                                         lhsT=Uff[:, c, tt * 128:(tt + 1) * 128],
                                         rhs=w2_sb[:, c, :],
                                         start=(c == 0), stop=(c == IFF // 128 - 1))
                    ot = work.tile([128, DIM], f32, tag="ot", name="ot")
                    nc.vector.tensor_add(out=ot, in0=resid[:, qs, 0:DIM], in1=po)
                    if "ff_b2" in flags:
                        nc.vector.tensor_add(out=ot, in0=ot, in1=bcast["ff_b2"])
                    nc.sync.dma_start(out=out_d[qs * 128:(qs + 1) * 128, :], in_=ot)

            # ============ emission schedule: the two q-halves are interleaved
            # within each phase so every finish/LN transpose's DMA latency is
            # covered by the other half's matmul stream.
            NB = S // 512
            for nb in range(NB):
                hb = work.tile([128, 4, DIMP], bf, tag="hblk", bufs=2, name="h1")
                for tt in range(4):
                    t = nb * 4 + tt
                    if t + 4 < S // 128:
                        load_xt(t + 4)
                    ln_into(hb[:, tt, :], xts.pop(t), "ln1_w", "ln1_b")
                transpose_blk(h1T[:, nb * 4:nb * 4 + 4, :, :], hb,
                              "scalar" if nb % 2 == 0 else "sync")
                for g in range(4):
                    kproj_item(nb, g)
                for tt in range(4):
                    vproj_item(nb * 4 + tt)
            acc = attn1_half(0)
            afm0 = finish_attn_norm(acc)          # frees acc banks for qh1
            # cross-attn/FF weights + context K,V hide under attn1(qh0)/(qh1)
            w2_sb, b1t, ctxT_sb = load_late_weights()
            cross_kv()
            acc = attn1_half(1)
            afm1 = finish_attn_norm(acc)
            finish_attn_proj(0, afm0, wsb["a1_Wo"], "a1_bo")
            ln_block(0, "ln2_w", "ln2_b")
            finish_attn_proj(1, afm1, wsb["a1_Wo"], "a1_bo")
            acc = attn2_half(0)
            afm0 = finish_attn_norm(acc)
            ln_block(1, "ln2_w", "ln2_b")
            finish_attn_proj(0, afm0, wsb["a2_Wo"], "a2_bo")
            acc = attn2_half(1)
            afm1 = finish_attn_norm(acc)
            ln_block(0, "ln3_w", "ln3_b")
            finish_attn_proj(1, afm1, wsb["a2_Wo"], "a2_bo")
            ff_inner(0)
            ln_block(1, "ln3_w", "ln3_b")
            ff_out(0)
            ff_inner(1)
            ff_out(1)

    nc.compile()
    return nc


_CACHE = {}


def _get_nc(S, R, flags):
    key = (S, R, tuple(sorted(flags)))
    if key not in _CACHE:
        _CACHE[key] = build_nc(S, R, flags)
    return _CACHE[key]


def _pad_qk8(w):
    """Self-attn Q/K weight layout for fp8 DoubleRow: per head h (g=h//4,
    m=h%4), sub i: block col 128*(2g+i) + 32*m + dk <- w col 40h + 20i + dk."""
    w = np.asarray(w)
    out = np.zeros((w.shape[0], 512), w.dtype)
    for h in range(HEADS):
        g, m = divmod(h, 4)
        for i in range(2):
            c0 = 128 * (2 * g + i) + 32 * m
            out[:, c0:c0 + 20] = w[:, DH * h + 20 * i:DH * h + 20 * i + 20]
    return out


def _pad_qk2(w, par=None):
    """Q/K layout: 2-head groups at partition offsets {0,64}. With par set,
    only even (par=0) or odd (par=1) heads are kept (others zero) so the
    score matmul can contract over all 128 partitions with one shared K."""
    w = np.asarray(w)
    out = np.zeros((w.shape[0], 512), w.dtype)
    for h in range(HEADS):
        g, j = divmod(h, 2)
        if par is not None and j != par:
            continue
        out[:, 128 * g + 64 * j:128 * g + 64 * j + DH] = w[:, DH * h:DH * h + DH]
    return out


def make_in_maps(x, context, ln_params, weights):
    """Host-side prep: returns (flags, in_maps, R, S, Bn)."""
    x = np.asarray(x)
    context = np.asarray(context)
    Bn = x.shape[0]
    S = x.shape[1]
    R = S * Bn // NCORES
    flags = set()
    for nm in ("ln1_w", "ln2_w", "ln3_w"):
        if not np.allclose(np.asarray(ln_params[nm]), 1.0):
            flags.add(nm)
    for nm in ("ln1_b", "ln2_b", "ln3_b", "a1_bo", "a2_bo", "ff_b2"):
        if not np.allclose(np.asarray(ln_params[nm]), 0.0):
            flags.add(nm)
    weights = dict(weights)
    w1q = weights.pop("a1_Wq")
    weights["a1_WqA"] = _pad_qk2(w1q, 0)
    weights["a1_WqB"] = _pad_qk2(w1q, 1)
    weights["a1_Wk"] = _pad_qk2(weights["a1_Wk"])
    w2q = weights.pop("a2_Wq")
    weights["a2_WqA"] = _pad_qk2(w2q, 0)
    weights["a2_WqB"] = _pad_qk2(w2q, 1)
    weights["a2_Wk"] = _pad_qk2(weights["a2_Wk"])
    shared = {nm: np.ascontiguousarray(np.asarray(w).astype(BF16))
              for nm, w in weights.items()}
    shared["ff_b1"] = np.ascontiguousarray(
        np.asarray(ln_params["ff_b1"]).astype(np.float32))
    for nm in flags:
        shared[nm] = np.ascontiguousarray(
            np.asarray(ln_params[nm]).astype(np.float32))
    pad = ((0, 0), (0, 0), (0, DIMP - DIM))
    xbf = np.ascontiguousarray(np.pad(x, pad).astype(BF16))
    ctxT = np.ascontiguousarray(np.asarray(context).astype(BF16).transpose(0, 2, 1))
    xf32 = np.ascontiguousarray(np.pad(x, pad).astype(np.float32))
    in_maps = []
    cpb = NCORES // Bn
    for core in range(NCORES):
        b, c = divmod(core, cpb)
        m = dict(shared)
        m["xfull"] = xbf[b]
        m["xq"] = np.ascontiguousarray(xf32[b, c * R:(c + 1) * R])
        m["ctxT"] = ctxT[b]
        in_maps.append(m)
    return flags, in_maps, R, S, Bn


def kernel(x, context, ln1_w, ln1_b, ln2_w, ln2_b, ln3_w, ln3_b,
           a1_Wq, a1_Wk, a1_Wv, a1_Wo, a1_bo,
           a2_Wq, a2_Wk, a2_Wv, a2_Wo, a2_bo,
           ff_W1, ff_b1, ff_W2, ff_b2, _trace=False):
    from concourse.bass_utils import run_bass_kernel_spmd

    weights = dict(a1_Wq=a1_Wq, a1_Wk=a1_Wk, a1_Wv=a1_Wv, a1_Wo=a1_Wo,
                   a2_Wq=a2_Wq, a2_Wk=a2_Wk, a2_Wv=a2_Wv, a2_Wo=a2_Wo,
                   ff_W1=ff_W1, ff_W2=ff_W2)
    ln_params = dict(ln1_w=ln1_w, ln1_b=ln1_b, ln2_w=ln2_w, ln2_b=ln2_b,
                     ln3_w=ln3_w, ln3_b=ln3_b, a1_bo=a1_bo, a2_bo=a2_bo,
                     ff_b1=ff_b1, ff_b2=ff_b2)
    flags, in_maps, R, S, Bn = make_in_maps(x, context, ln_params, weights)
    nc = _get_nc(S, R, flags)
    res = run_bass_kernel_spmd(nc, in_maps, core_ids=list(range(NCORES)),
                               trace=_trace)
    out = np.empty((Bn, S, DIM), np.float32)
    cpb = NCORES // Bn
    for core in range(NCORES):
        b, c = divmod(core, cpb)
        out[b, c * R:(c + 1) * R] = res.results[core]["out"]
    kernel._last_result = res
    return out

